# revision 1
# baseline (speedup 1.0000x reference)
"""Trainium2 Bass kernel for a 3-layer GCN encoder with global max pool.

Strategy (8 NeuronCores, SPMD, 5 launches, host staging between launches):
  - Nodes are partitioned graph-wise (graphs g -> core g//64). Between
    launches the host only MOVES device-computed values (concat / permute /
    replicate rows into padded layouts) - every FLOP of the network
    (matmuls, aggregation sums, scaling, bias, relu, max pool) runs on
    device.
  - Everything is laid out TRANSPOSED (features on partitions, nodes on the
    free dim, two node-columns per 128 partitions). All matmuls then use
    fixed weights (lhsT=W) over 512-column chunks - no PE transposes.
  - GCN normalization is factored: out = s * Agg(s * h), s = 1/sqrt(deg),
    with the self-loop folded in as slot 0 of each node's message list.
  - Aggregation per layer: the host stages the per-edge messages (rows of
    the previous layer's device-computed table, fp16) into a padded
    [128, cols*D] tensor; the device bulk-loads it (~360 GB/s, no per-edge
    DMA descriptors) and pairwise-tree-sums the D slot axis on DVE.
  - Launches: L1  T1 = s*(X@W1)
              L2  T2 = s*relu(s*Agg(T1) + b1)
              L3  T3 = s*(relu((s*Agg(T2))@W2 + b2)@W3)
              L4  H3 = s*Agg(T3) + b3
              L5  per-graph max pool over H3 (graph-grouped staged layout)
"""

import numpy as np

N = 50000
IN_DIM = 128
HID = 64
F2 = 2 * HID
N_GRAPHS = 512
C = 8
P = 128
GPC = N_GRAPHS // C
CG = 64          # columns (node pairs) per reduce group
CH = 512         # matmul column chunk
F16 = np.float16


# --------------------------------------------------------------------------
# Host-side preprocessing (graph structure only - no feature arithmetic)
# --------------------------------------------------------------------------

def _host_prep(edge_index, batch):
    src = np.asarray(edge_index[0], dtype=np.int64)
    dst = np.asarray(edge_index[1], dtype=np.int64)
    batch = np.asarray(batch, dtype=np.int64)
    core_of = batch // GPC

    indeg = np.bincount(dst, minlength=N)
    k = indeg + 1                     # slots per node incl. self loop
    s = (1.0 / np.sqrt(k.astype(np.float64))).astype(np.float32)

    # in-neighbor lists grouped by dst
    eorder = np.argsort(dst, kind="stable")
    esrc = src[eorder]
    estart = np.zeros(N + 1, np.int64)
    np.cumsum(np.bincount(dst, minlength=N), out=estart[1:])

    # per-core node order: descending k (big blocks first), paired (2i, 2i+1)
    orders = []
    for c in range(C):
        nodes = np.nonzero(core_of == c)[0]
        orders.append(nodes[np.argsort(-k[nodes], kind="stable")])
    ncols_c = [(-(-len(o) // 2)) for o in orders]
    NCOL = max(ncols_c)
    ngroups = -(-NCOL // CG)

    # per-core tops/bottoms (matmul/storage order), padded with -1
    tops = np.full((C, NCOL), -1, np.int64)
    bots = np.full((C, NCOL), -1, np.int64)
    for c in range(C):
        o = orders[c]
        tops[c, : len(o[0::2])] = o[0::2]
        bots[c, : len(o[1::2])] = o[1::2]

    kk = np.concatenate([k, [0]])     # k of node, 0 for -1 pad (via index N)
    topsx = np.where(tops >= 0, tops, N)
    botsx = np.where(bots >= 0, bots, N)
    colk = np.maximum(kk[topsx], kk[botsx])       # [C, NCOL]

    D_g = np.zeros(ngroups, np.int64)
    for g in range(ngroups):
        D_g[g] = max(1, int(colk[:, g * CG : (g + 1) * CG].max()))
    off_g = np.zeros(ngroups + 1, np.int64)
    np.cumsum([int(D_g[g]) * min(CG, NCOL - g * CG) for g in range(ngroups)],
              out=off_g[1:])
    SLOTS = int(off_g[-1])

    # merge adjacent equal-D groups into raw blocks over the desc-k order
    raw = []
    g = 0
    while g < ngroups:
        g2 = g
        while g2 + 1 < ngroups and D_g[g2 + 1] == D_g[g]:
            g2 += 1
        col0 = g * CG
        ncols = min((g2 + 1) * CG, NCOL) - col0
        raw.append((col0, ncols, int(D_g[g])))
        g = g2 + 1

    # schedule blocks small-first / small-last so both the load pipeline
    # primes quickly and the compute tail drains quickly; permute columns
    # to make the scheduled order the storage order.
    work = [nc_ * max(D - 1, 1) for (_, nc_, D) in raw]
    asc = sorted(range(len(raw)), key=lambda i: work[i])
    sched = asc[0:2] + sorted(asc[3:], key=lambda i: -work[i]) + asc[2:3]
    col_perm = np.concatenate(
        [np.arange(raw[i][0], raw[i][0] + raw[i][1]) for i in sched])
    tops = tops[:, col_perm]
    bots = bots[:, col_perm]
    topsx = np.where(tops >= 0, tops, N)
    botsx = np.where(bots >= 0, bots, N)

    blocks = []
    col0, off = 0, 0
    for i in sched:
        ncols, D = raw[i][1], raw[i][2]
        nsplit = max(1, -(-(ncols * D) // 99999))
        step = -(-(ncols // nsplit) // 2) * 2    # even split points
        for a in range(0, ncols, step):
            w = min(step, ncols - a)
            blocks.append((col0, w, D, off))
            col0 += w
            off += w * D
    SLOTS = off

    # slot -> source node maps (N = zero row) for tops/bottoms
    srcmap = np.full((C, 2, SLOTS), N, np.int64)
    for c in range(C):
        for (col0, ncols, D, off) in blocks:
            for half, nodes_h in ((0, topsx[c]), (1, botsx[c])):
                cols = nodes_h[col0 : col0 + ncols]
                for i, n in enumerate(cols):
                    if n == N:
                        continue
                    base = off + i * D
                    srcmap[c, half, base] = n          # self
                    e0, e1 = estart[n], estart[n + 1]
                    srcmap[c, half, base + 1 : base + 1 + (e1 - e0)] = esrc[e0:e1]

    # s replicated strips (fp16): SR [128, NCOL], halves [64, NCOL]
    sx = np.concatenate([s, [0.0]]).astype(F16)
    SRE = sx[topsx][:, None, :].repeat(HID, axis=1)   # [C, 64, NCOL]
    SRO = sx[botsx][:, None, :].repeat(HID, axis=1)
    SR = np.concatenate([SRE, SRO], axis=1)           # [C, 128, NCOL]

    # pooling: graph-grouped paired layout
    gl = batch % GPC
    cnt = np.zeros((C, GPC), np.int64)
    np.add.at(cnt, (core_of, gl), 1)
    S2 = int(-(-cnt.max() // 2))
    poolmap = np.full((C, 2, GPC * S2), N, np.int64)
    for c in range(C):
        for g in range(GPC):
            nodes = np.nonzero((core_of == c) & (gl == g))[0]
            e = nodes[0::2]
            o = nodes[1::2]
            poolmap[c, 0, g * S2 : g * S2 + len(e)] = e
            poolmap[c, 1, g * S2 : g * S2 + len(o)] = o

    meta = dict(NCOL=NCOL, ngroups=ngroups, D_g=[int(x) for x in D_g],
                off_g=[int(x) for x in off_g], SLOTS=SLOTS, S2=S2,
                blocks=blocks)
    return dict(meta=meta, orders=orders, tops=tops, bots=bots,
                topsx=topsx, botsx=botsx, srcmap=srcmap, poolmap=poolmap,
                SR=SR, SRE=SRE, SRO=SRO, cnt=cnt)


# --------------------------------------------------------------------------
# Bass programs
# --------------------------------------------------------------------------

def _mk_bass():
    import concourse.bacc as bacc
    return bacc.Bacc(None)


def _tree_reduce_into(nc, mg, out_ap, ncols, D, op, eng=None):
    """Pairwise reduce [128, ncols, D] over D; final level fused into
    out_ap [128, ncols]. Tails folded early so every halving add has h>=2."""
    if eng is None:
        eng = nc.vector
    v = mg.rearrange("p (c d) -> p c d", d=D)
    cur = D
    while cur > 2:
        if cur % 2:
            eng.tensor_tensor(
                out=v[:, :, 0:1], in0=v[:, :, 0:1],
                in1=v[:, :, cur - 1 : cur], op=op)
            cur -= 1
        h = cur // 2
        eng.tensor_tensor(
            out=v[:, :, 0:h], in0=v[:, :, 0:h], in1=v[:, :, h : 2 * h], op=op)
        cur = h
    if cur == 2:
        eng.tensor_tensor(out=out_ap, in0=v[:, :, 0], in1=v[:, :, 1], op=op)
    else:
        eng.tensor_copy(out_ap, v[:, :, 0])


_POOL_OFFLOAD = True


def _block_engine_picker(nc, post_ops=3, total_elems=None, cutoff=0.72):
    """Greedy DVE/Pool balance: each block's tree + its post-ops run wholly
    on one engine (0.53 vs ~2.0 ns/elem). Pool only takes work in the first
    part of the stream so its slower pipeline never becomes the tail."""
    state = {"dve": 0.0, "pool": 0.0, "seen": 0.0}

    def pick(ncols, D):
        elems = ncols * max(D - 1, 1) + post_ops * ncols
        state["seen"] += elems
        early = total_elems is None or state["seen"] < cutoff * total_elems
        if _POOL_OFFLOAD and early and (
                state["pool"] + elems * 2.0 < state["dve"] + elems * 0.53):
            state["pool"] += elems * 2.0
            return nc.gpsimd
        state["dve"] += elems * 0.53
        return nc.vector

    return pick


LG_SLOTS = 4000     # slots per load-group (~1 MB per DMA)


def _agg_blocks(nc, meta, MSG_d, A, msgp, post_fn, first_loads=None,
                post_ops=3, cutoff=0.72):
    """Pipelined: load groups of blocks with one DMA each, tree-reduce each
    block on DVE or Pool, then run post_fn(col0, ncols, eng) on the SAME
    engine so the two pipelines stay independent."""
    import concourse.mybir as mybir
    f16 = mybir.dt.float16
    Alu = mybir.AluOpType
    groups = []
    cur, slots = [], 0
    for b in meta["blocks"]:
        bslots = b[1] * b[2]
        cap = 1200 if len(groups) < 2 else LG_SLOTS
        if cur and slots + bslots > cap:
            groups.append((cur, slots))
            cur, slots = [], 0
        cur.append(b)
        slots += bslots
    if cur:
        groups.append((cur, slots))
    LMAX = max(s for _, s in groups)
    total = sum(b[1] * max(b[2] - 1, 1) + post_ops * b[1]
                for b in meta["blocks"])
    pick = _block_engine_picker(nc, post_ops, total, cutoff)
    for gi, (blks, slots) in enumerate(groups):
        mg = msgp.tile([P, LMAX], f16, tag="msg")
        off0 = blks[0][3]
        nc.sync.dma_start(mg[:, :slots], MSG_d[:, off0 : off0 + slots])
        if gi == 0 and first_loads is not None:
            first_loads()
        for (col0, ncols, D, off) in blks:
            sl = mg[:, off - off0 : off - off0 + ncols * D]
            eng = pick(ncols, D)
            _tree_reduce_into(nc, sl, A[:, col0 : col0 + ncols],
                              ncols, D, Alu.add, eng=eng)
            post_fn(col0, ncols, eng)


def _prog_l1(meta):
    """T1 = s * (X @ W1), stored as two [64, NCOL] half strips."""
    import concourse.mybir as mybir
    import concourse.tile as tile
    f16 = mybir.dt.float16
    f32 = mybir.dt.float32
    Alu = mybir.AluOpType
    NCOL = meta["NCOL"]
    nc = _mk_bass()

    XT_d = nc.dram_tensor("XT", [IN_DIM, 2 * NCOL], f16, kind="ExternalInput")
    W1_d = nc.dram_tensor("W1", [IN_DIM, HID], f16, kind="ExternalInput")
    SRE_d = nc.dram_tensor("SRE", [HID, NCOL], f16, kind="ExternalInput")
    SRO_d = nc.dram_tensor("SRO", [HID, NCOL], f16, kind="ExternalInput")
    T1E_d = nc.dram_tensor("T1E", [HID, NCOL], f16, kind="ExternalOutput")
    T1O_d = nc.dram_tensor("T1O", [HID, NCOL], f16, kind="ExternalOutput")

    with tile.TileContext(nc, num_cores=C) as tc:
        with (
            tc.tile_pool(name="const", bufs=1) as const,
            tc.tile_pool(name="ps", bufs=4, space="PSUM") as psp,
        ):
            W1_s = const.tile([IN_DIM, HID], f16)
            nc.sync.dma_start(W1_s[:], W1_d[:])
            SRE_s = const.tile([HID, NCOL], f16)
            SRO_s = const.tile([HID, NCOL], f16)
            T1E_s = const.tile([HID, NCOL], f16)
            T1O_s = const.tile([HID, NCOL], f16)
            NQ = -(-NCOL // CH)
            XQ = [const.tile([IN_DIM, CH], f16, name=f"xq{half}_{q}",
                             tag=f"xq{half}_{q}")
                  for half in range(2) for q in range(NQ)]

            def load_xq(i):
                half, q = divmod(i, NQ)
                a = q * CH
                w = min(CH, NCOL - a)
                nc.sync.dma_start(
                    XQ[i][:, :w],
                    XT_d[:, half * NCOL + a : half * NCOL + a + w])

            load_xq(0)
            load_xq(1)
            nc.sync.dma_start(SRE_s[:], SRE_d[:])
            nc.sync.dma_start(SRO_s[:], SRO_d[:])
            for i in range(2, 2 * NQ):
                load_xq(i)
            for half, (T_s, SR_s, T_d) in enumerate(
                    ((T1E_s, SRE_s, T1E_d), (T1O_s, SRO_s, T1O_d))):
                stored = 0
                for q, a in enumerate(range(0, NCOL, CH)):
                    w = min(CH, NCOL - a)
                    ps = psp.tile([HID, CH], f32, tag="ps")
                    nc.tensor.matmul(
                        ps[:, :w], lhsT=W1_s[:], rhs=XQ[half * NQ + q][:, :w],
                        start=True, stop=True)
                    nc.vector.tensor_tensor(
                        out=T_s[:, a : a + w], in0=ps[:, :w],
                        in1=SR_s[:, a : a + w], op=Alu.mult)
                    if q % 3 == 2 or a + w == NCOL:
                        nc.scalar.dma_start(T_d[:, stored : a + w],
                                            T_s[:, stored : a + w])
                        stored = a + w
    nc.compile()
    return nc


def _prog_agg(meta, layer):
    """L2 (layer==1): T2 = s*relu(s*A + b1)        -> OUT [128, NCOL]
       L4 (layer==3): H3 = s*A + b3                -> OUT [128, NCOL]"""
    import concourse.mybir as mybir
    import concourse.tile as tile
    f16 = mybir.dt.float16
    f32 = mybir.dt.float32
    Alu = mybir.AluOpType
    Act = mybir.ActivationFunctionType
    NCOL, SLOTS = meta["NCOL"], meta["SLOTS"]
    nc = _mk_bass()

    MSG_d = nc.dram_tensor("MSG", [P, SLOTS], f16, kind="ExternalInput")
    SR_d = nc.dram_tensor("SR", [P, NCOL], f16, kind="ExternalInput")
    B_d = nc.dram_tensor("B", [P, 1], f32, kind="ExternalInput")
    OUT_d = nc.dram_tensor("OUT", [P, NCOL], f16, kind="ExternalOutput")

    with tile.TileContext(nc, num_cores=C) as tc:
        with (
            tc.tile_pool(name="const", bufs=1) as const,
            tc.tile_pool(name="msg", bufs=8) as msgp,
        ):
            SR_s = const.tile([P, NCOL], f16)
            B_s = const.tile([P, 1], f32)
            A = const.tile([P, NCOL], f16)
            OUT_s = const.tile([P, NCOL], f16)

            def first_loads():
                nc.sync.dma_start(SR_s[:], SR_d[:])
                nc.sync.dma_start(B_s[:], B_d[:])

            def post_fn(col0, ncols, eng):
                cols = slice(col0, col0 + ncols)
                eng.tensor_tensor(out=A[:, cols], in0=A[:, cols],
                                  in1=SR_s[:, cols], op=Alu.mult)
                if layer == 1:
                    eng.tensor_scalar(out=A[:, cols], in0=A[:, cols],
                                      scalar1=B_s[:], scalar2=0.0,
                                      op0=Alu.add, op1=Alu.max)
                    eng.tensor_tensor(out=OUT_s[:, cols], in0=A[:, cols],
                                      in1=SR_s[:, cols], op=Alu.mult)
                else:
                    eng.tensor_scalar(out=OUT_s[:, cols], in0=A[:, cols],
                                      scalar1=B_s[:], scalar2=None,
                                      op0=Alu.add)
                nc.scalar.dma_start(OUT_d[:, cols], OUT_s[:, cols])

            _agg_blocks(nc, meta, MSG_d, A, msgp, post_fn, first_loads,
                        post_ops=3 if layer == 1 else 2)
    nc.compile()
    return nc


def _prog_l3(meta):
    """T3 = s * (relu((s*A) @ W2 + b2) @ W3), two [64, NCOL] half strips."""
    import concourse.mybir as mybir
    import concourse.tile as tile
    f16 = mybir.dt.float16
    f32 = mybir.dt.float32
    Alu = mybir.AluOpType
    Act = mybir.ActivationFunctionType
    NCOL, SLOTS = meta["NCOL"], meta["SLOTS"]
    nc = _mk_bass()

    MSG_d = nc.dram_tensor("MSG", [P, SLOTS], f16, kind="ExternalInput")
    SR_d = nc.dram_tensor("SR", [P, NCOL], f16, kind="ExternalInput")
    SRE_d = nc.dram_tensor("SRE", [HID, NCOL], f16, kind="ExternalInput")
    SRO_d = nc.dram_tensor("SRO", [HID, NCOL], f16, kind="ExternalInput")
    W2_d = nc.dram_tensor("W2", [P, F2], f16, kind="ExternalInput")
    W3_d = nc.dram_tensor("W3", [F2, HID], f16, kind="ExternalInput")
    B2_d = nc.dram_tensor("B2", [F2, 1], f32, kind="ExternalInput")
    T3E_d = nc.dram_tensor("T3E", [HID, NCOL], f16, kind="ExternalOutput")
    T3O_d = nc.dram_tensor("T3O", [HID, NCOL], f16, kind="ExternalOutput")

    with tile.TileContext(nc, num_cores=C) as tc:
        with (
            tc.tile_pool(name="const", bufs=1) as const,
            tc.tile_pool(name="msg", bufs=4) as msgp,
            tc.tile_pool(name="ps2", bufs=3, space="PSUM") as ps2p,
            tc.tile_pool(name="ps3", bufs=3, space="PSUM") as ps3p,
        ):
            SR_s = const.tile([P, NCOL], f16)
            SRE_s = const.tile([HID, NCOL], f16)
            SRO_s = const.tile([HID, NCOL], f16)
            W2_s = const.tile([P, F2], f16)
            W3_s = const.tile([F2, HID], f16)
            B2_s = const.tile([F2, 1], f32)

            A = const.tile([P, NCOL], f16)
            T3E_s = const.tile([HID, NCOL], f16)
            T3O_s = const.tile([HID, NCOL], f16)
            H2E = const.tile([F2, NCOL], f16)
            H2O = const.tile([F2, NCOL], f16)

            def chunk_flow(a, w):
                halves = ((A[0:HID, a : a + w], W2_s[0:HID, :], (0, 0),
                           H2E, SRE_s, T3E_s, T3E_d),
                          (A[HID:P, a : a + w], W2_s[HID:P, :], (HID, 0),
                           H2O, SRO_s, T3O_s, T3O_d))
                ps2s, ps3s = [], []
                for (rhs, lhsT2, tp, H2, SRh, T_s, T_d) in halves:
                    ps2 = ps2p.tile([F2, CH], f32, tag="ps2")
                    nc.tensor.matmul(ps2[:, :w], lhsT=lhsT2, rhs=rhs,
                                     start=True, stop=True, tile_position=tp)
                    ps2s.append(ps2)
                for ps2, (rhs, lhsT2, tp, H2, SRh, T_s, T_d) in zip(ps2s, halves):
                    nc.scalar.activation(out=H2[:, a : a + w], in_=ps2[:, :w],
                                         func=Act.Relu, bias=B2_s[:], scale=1.0)
                for (rhs, lhsT2, tp, H2, SRh, T_s, T_d) in halves:
                    ps3 = ps3p.tile([HID, CH], f32, tag="ps3")
                    nc.tensor.matmul(ps3[:, :w], lhsT=W3_s[:],
                                     rhs=H2[:, a : a + w], start=True, stop=True)
                    ps3s.append(ps3)
                for ps3, (rhs, lhsT2, tp, H2, SRh, T_s, T_d) in zip(ps3s, halves):
                    nc.vector.tensor_tensor(
                        out=T_s[:, a : a + w], in0=ps3[:, :w],
                        in1=SRh[:, a : a + w], op=Alu.mult)
                    nc.scalar.dma_start(T_d[:, a : a + w], T_s[:, a : a + w])

            def first_loads():
                nc.sync.dma_start(SR_s[:], SR_d[:])
                nc.sync.dma_start(SRE_s[:], SRE_d[:])
                nc.sync.dma_start(SRO_s[:], SRO_d[:])
                nc.sync.dma_start(W2_s[:], W2_d[:])
                nc.sync.dma_start(W3_s[:], W3_d[:])
                nc.sync.dma_start(B2_s[:], B2_d[:])

            state = {"next": 0}
            covered = np.zeros(NCOL, bool)

            def post_fn(col0, ncols, eng):
                cols = slice(col0, col0 + ncols)
                eng.tensor_tensor(out=A[:, cols], in0=A[:, cols],
                                  in1=SR_s[:, cols], op=Alu.mult)
                covered[cols] = True
                while (state["next"] < NCOL
                       and covered[state["next"]
                                   : min(state["next"] + CH, NCOL)].all()):
                    w = min(CH, NCOL - state["next"])
                    chunk_flow(state["next"], w)
                    state["next"] += w

            _agg_blocks(nc, meta, MSG_d, A, msgp, post_fn, first_loads,
                        post_ops=1, cutoff=0.5)
    nc.compile()
    return nc


def _prog_pool(meta):
    """Per-graph max over staged [128, GPC*S2] fp16; OUT [64, GPC] fp32
    is graphs x features (host transposes)."""
    import concourse.mybir as mybir
    import concourse.tile as tile
    from concourse.masks import make_identity
    f16 = mybir.dt.float16
    f32 = mybir.dt.float32
    Alu = mybir.AluOpType
    S2 = meta["S2"]
    nc = _mk_bass()

    MSGP_d = nc.dram_tensor("MSGP", [P, GPC * S2], f16, kind="ExternalInput")
    OUT_d = nc.dram_tensor("OUT", [GPC, HID], f32, kind="ExternalOutput")

    with tile.TileContext(nc, num_cores=C) as tc:
        with (
            tc.tile_pool(name="const", bufs=1) as const,
            tc.tile_pool(name="ps", bufs=2, space="PSUM") as psp,
        ):
            mg = const.tile([P, GPC * S2], f16)
            PM = const.tile([P, GPC], f16)
            GC = 16
            for g0 in range(0, GPC, GC):
                sl = slice(g0 * S2, (g0 + GC) * S2)
                nc.sync.dma_start(mg[:, sl], MSGP_d[:, sl])
                _tree_reduce_into(nc, mg[:, sl], PM[:, g0 : g0 + GC],
                                  GC, S2, Alu.max)
            ident = const.tile([P, P], f16)
            make_identity(nc, ident[:])
            psT = psp.tile([GPC, P], f16, tag="t")
            nc.tensor.transpose(psT[:], PM[:], ident[:])
            sT = const.tile([GPC, P], f16)
            nc.vector.tensor_copy(sT[:], psT[:])
            OUT_s = const.tile([GPC, HID], f32)
            nc.vector.tensor_tensor(out=OUT_s[:], in0=sT[:, 0:HID],
                                    in1=sT[:, HID:P], op=Alu.max)
            nc.sync.dma_start(OUT_d[:], OUT_s[:])
    nc.compile()
    return nc


# --------------------------------------------------------------------------
# Entry point
# --------------------------------------------------------------------------

_RUN_KWARGS = {}
_EXEC_NS = []
_PROFILE = False


def _stage_msgs(T_full, srcmap_c):
    """[N+1, HID] table + [2, SLOTS] slot->row map -> [128, SLOTS] fp16."""
    top = T_full[srcmap_c[0]].T      # [64, SLOTS]
    bot = T_full[srcmap_c[1]].T
    return np.ascontiguousarray(np.concatenate([top, bot], axis=0))


def _assemble(prep, parts_E, parts_O):
    """Per-core [64, NCOL] half strips -> full [N+1, HID] fp16 table."""
    T_full = np.zeros((N + 1, HID), F16)
    for c in range(C):
        tops, bots = prep["tops"][c], prep["bots"][c]
        mE, mO = tops >= 0, bots >= 0
        T_full[tops[mE]] = parts_E[c][:, mE].T
        T_full[bots[mO]] = parts_O[c][:, mO].T
    return T_full


def kernel(data, edge_index, batch, W1, b1, W2, b2, W3, b3):
    from concourse.bass_utils import run_bass_kernel_spmd

    data = np.asarray(data, dtype=np.float32)
    edge_index = np.asarray(edge_index, dtype=np.int32)
    batch_np = np.asarray(batch, dtype=np.int32)

    prep = _host_prep(edge_index, batch_np)
    meta = prep["meta"]
    NCOL = meta["NCOL"]

    W1f = np.asarray(W1, np.float32).astype(F16)            # [128, 64]
    W2f = np.asarray(W2, np.float32).astype(F16)            # [64, 128]
    W3f = np.asarray(W3, np.float32).astype(F16)            # [128, 64]
    B1r = np.tile(np.asarray(b1, np.float32), 2)[:, None].copy()
    B2r = np.asarray(b2, np.float32)[:, None].copy()
    B3r = np.tile(np.asarray(b3, np.float32), 2)[:, None].copy()

    Xx = np.concatenate([data, np.zeros((1, IN_DIM), np.float32)], axis=0)
    XT = np.empty((C, IN_DIM, 2 * NCOL), F16)
    for c in range(C):
        XT[c, :, :NCOL] = Xx[prep["topsx"][c]].T
        XT[c, :, NCOL:] = Xx[prep["botsx"][c]].T

    cores = list(range(C))
    del _EXEC_NS[:]

    def run(nc, in_maps):
        if _PROFILE:
            from concourse.timeline_sim import TimelineSim
            _EXEC_NS.append(TimelineSim(nc, require_finite=False).simulate())
        res = run_bass_kernel_spmd(nc, in_maps, cores, **_RUN_KWARGS)
        if res.exec_time_ns is not None:
            _EXEC_NS.append(res.exec_time_ns)
        return res.results

    # ---- L1: T1 = s * (X @ W1) ----
    r1 = run(_prog_l1(meta),
             [{"XT": np.ascontiguousarray(XT[c]), "W1": W1f,
               "SRE": np.ascontiguousarray(prep["SRE"][c]),
               "SRO": np.ascontiguousarray(prep["SRO"][c])}
              for c in range(C)])
    T1 = _assemble(prep,
                   [np.asarray(r1[c]["T1E"]) for c in range(C)],
                   [np.asarray(r1[c]["T1O"]) for c in range(C)])

    # ---- L2: T2 = s*relu(s*Agg(T1) + b1) ----
    r2 = run(_prog_agg(meta, 1),
             [{"MSG": _stage_msgs(T1, prep["srcmap"][c]),
               "SR": np.ascontiguousarray(prep["SR"][c]), "B": B1r}
              for c in range(C)])
    T2 = _assemble(prep,
                   [np.asarray(r2[c]["OUT"])[0:HID] for c in range(C)],
                   [np.asarray(r2[c]["OUT"])[HID:P] for c in range(C)])

    # ---- L3: T3 = s*(relu((s*Agg(T2))@W2 + b2)@W3) ----
    r3 = run(_prog_l3(meta),
             [{"MSG": _stage_msgs(T2, prep["srcmap"][c]),
               "SR": np.ascontiguousarray(prep["SR"][c]),
               "SRE": np.ascontiguousarray(prep["SRE"][c]),
               "SRO": np.ascontiguousarray(prep["SRO"][c]),
               "W2": np.concatenate([W2f, W2f], axis=0), "W3": W3f, "B2": B2r}
              for c in range(C)])
    T3 = _assemble(prep,
                   [np.asarray(r3[c]["T3E"]) for c in range(C)],
                   [np.asarray(r3[c]["T3O"]) for c in range(C)])

    # ---- L4: H3 = s*Agg(T3) + b3 ----
    r4 = run(_prog_agg(meta, 3),
             [{"MSG": _stage_msgs(T3, prep["srcmap"][c]),
               "SR": np.ascontiguousarray(prep["SR"][c]), "B": B3r}
              for c in range(C)])
    H3 = _assemble(prep,
                   [np.asarray(r4[c]["OUT"])[0:HID] for c in range(C)],
                   [np.asarray(r4[c]["OUT"])[HID:P] for c in range(C)])
    H3[N] = np.float16(-60000.0)     # pad row for the pool staging

    # ---- L5: per-graph max pool ----
    r5 = run(_prog_pool(meta),
             [{"MSGP": _stage_msgs(H3, prep["poolmap"][c])}
              for c in range(C)])
    out = np.concatenate([np.asarray(r5[c]["OUT"]) for c in range(C)],
                         axis=0).astype(np.float32)
    out[prep["cnt"].reshape(-1) == 0] = -np.inf
    return out



# revision 5
# speedup vs baseline: 1.3555x; 1.3555x over previous
"""Trainium2 Bass kernel for a 3-layer GCN encoder with global max pool.

Strategy (8 NeuronCores, SPMD, 5 launches, host staging between launches):
  - Nodes partitioned graph-wise (graph g -> core g//64). The host only MOVES
    device-computed bytes between launches (gather rows into padded message
    tables); every FLOP runs on device.
  - Aggregation layers stage per-edge messages in fp8 (e3m4) with fixed
    power-free scale factors folded into the device-side s-tables, halving
    HBM traffic vs fp16.
  - The aggregation sum runs on the Tensor engine: identity-weight matmuls
    accumulate message strips into PSUM (start/stop prefix accumulation).
    Columns (node pairs) are sorted by descending in-degree so the set of
    columns with a message at depth d is a prefix; strips are stored
    region-major (one 512-column PSUM bank region at a time) so PSUM holds
    each region until its sum completes.
  - Post-ops per region: DVE multiplies PSUM by the s-table; Activation
    applies (scaled) bias+relu; DVE writes the next layer's pre-scaled
    fp8 table directly.
  - Launches: L1  T1 = (s*M1) * (X @ W1)                 [fp8 out]
              L2  T2 = (s*M2) * relu(s*Agg(T1)/M1 + b1)  [fp8 out]
              L3  T3 = (s*M3) * (relu((s*Agg(T2)/M2)@W2 + b2)@W3)
              L4  H3 = s*Agg(T3)/M3 + b3                 [fp16 out]
              L5  per-graph max pool over H3 (depth-major staged layout)
"""

import numpy as np
import ml_dtypes

N = 50000
IN_DIM = 128
HID = 64
F2 = 2 * HID
N_GRAPHS = 512
C = 8
P = 128
GPC = N_GRAPHS // C
RW = 512            # psum region width (columns)
F16 = np.float16
F8 = ml_dtypes.float8_e3m4

M1, M2, M3 = 5.0, 11.0, 44.0   # staging scale factors (fold into s-tables)


# --------------------------------------------------------------------------
# Host-side preprocessing (graph structure only - no feature arithmetic)
# --------------------------------------------------------------------------

def _host_prep(edge_index, batch):
    src = np.asarray(edge_index[0], dtype=np.int64)
    dst = np.asarray(edge_index[1], dtype=np.int64)
    batch = np.asarray(batch, dtype=np.int64)
    core_of = batch // GPC

    indeg = np.bincount(dst, minlength=N)
    k = indeg + 1                     # slots per node incl. self loop
    s = (1.0 / np.sqrt(k.astype(np.float64))).astype(np.float32)

    # in-neighbor lists grouped by dst
    eorder = np.argsort(dst, kind="stable")
    esrc = src[eorder]
    estart = np.zeros(N + 1, np.int64)
    np.cumsum(np.bincount(dst, minlength=N), out=estart[1:])

    # per-core node order: descending k, paired (2i, 2i+1) into columns
    orders = []
    for c in range(C):
        nodes = np.nonzero(core_of == c)[0]
        orders.append(nodes[np.argsort(-k[nodes], kind="stable")])
    NCOL = max((len(o) + 1) // 2 for o in orders)

    tops = np.full((C, NCOL), -1, np.int64)
    bots = np.full((C, NCOL), -1, np.int64)
    for c in range(C):
        o = orders[c]
        tops[c, : len(o[0::2])] = o[0::2]
        bots[c, : len(o[1::2])] = o[1::2]
    topsx = np.where(tops >= 0, tops, N)
    botsx = np.where(bots >= 0, bots, N)

    # column depth = max over cores of max(k_top, k_bot); >=1 (self loop)
    kk = np.concatenate([k, [0]])
    D_col = np.maximum(kk[topsx], kk[botsx]).max(axis=0)
    D_col = np.maximum(D_col, 1)
    assert (np.diff(D_col) <= 0).all()
    DMAX = int(D_col[0])
    n_d = np.array([(D_col > d).sum() for d in range(DMAX)], np.int64)

    # region-major strips: (region_col0, d, w, slot_off)
    strips = []
    off = 0
    regions = []          # (col0, width, D_r, [strip indices])
    for r0 in range(0, NCOL, RW):
        wr = min(RW, NCOL - r0)
        Dr = int(D_col[r0])
        idxs = []
        for d in range(Dr):
            w = int(min(n_d[d] - r0, wr))
            assert w > 0
            idxs.append(len(strips))
            strips.append((r0, d, w, off))
            off += w
        regions.append((r0, wr, Dr, idxs))
    SLOTS = off

    # slot -> source node maps (N = zero row) for tops/bottoms
    indegx = np.concatenate([indeg, [0]])
    estartx = np.concatenate([estart[:-1], [0]])
    srcmap = np.full((C, 2, SLOTS), N, np.int64)
    for (r0, d, w, soff) in strips:
        for c in range(C):
            for half, nodes_h in ((0, topsx[c]), (1, botsx[c])):
                v = nodes_h[r0 : r0 + w]
                if d == 0:
                    srcmap[c, half, soff : soff + w] = v
                else:
                    sel = (d <= indegx[v]) & (v < N)
                    tgt = srcmap[c, half, soff : soff + w]
                    tgt[sel] = esrc[estartx[v[sel]] + d - 1]

    # s tables [C, 128, NCOL] fp16: top s in rows 0:64, bot s in rows 64:128
    def s_table(scale):
        sx = np.concatenate([s * scale, [0.0]]).astype(F16)
        top = sx[topsx][:, None, :].repeat(HID, axis=1)
        bot = sx[botsx][:, None, :].repeat(HID, axis=1)
        return np.concatenate([top, bot], axis=1)      # [C, 128, NCOL]

    SRT1 = s_table(M1)
    SA2 = s_table(1.0 / M1)
    SA3 = s_table(1.0 / M2)
    SA4 = s_table(1.0 / M3)

    # pooling: depth-major graph-grouped layout [128, S2*GPC]
    gl = batch % GPC
    cnt = np.zeros((C, GPC), np.int64)
    np.add.at(cnt, (core_of, gl), 1)
    S2 = int(-(-cnt.max() // 2))
    poolmap = np.full((C, 2, S2 * GPC), N, np.int64)
    for c in range(C):
        for g in range(GPC):
            nodes = np.nonzero((core_of == c) & (gl == g))[0]
            e = nodes[0::2]
            o = nodes[1::2]
            poolmap[c, 0, g : g + S2 * GPC : GPC][: len(e)] = e
            poolmap[c, 1, g : g + S2 * GPC : GPC][: len(o)] = o

    meta = dict(NCOL=NCOL, SLOTS=SLOTS, S2=S2, strips=strips,
                regions=regions)
    return dict(meta=meta, tops=tops, bots=bots, topsx=topsx, botsx=botsx,
                srcmap=srcmap, poolmap=poolmap, cnt=cnt,
                SRT1=SRT1, SA2=SA2, SA3=SA3, SA4=SA4)


# --------------------------------------------------------------------------
# Bass programs
# --------------------------------------------------------------------------

def _mk_bass():
    import concourse.bacc as bacc
    return bacc.Bacc(None)


def _msg_chunks(strips, SLOTS, first=1400, later=3600):
    """Split the slot axis into DMA chunks at strip boundaries."""
    cuts = []
    target = first
    for (r0, d, w, soff) in strips:
        end = soff + w
        if end >= target:
            cuts.append(end)
            target = end + later
    if not cuts or cuts[-1] != SLOTS:
        cuts.append(SLOTS)
    out = []
    a = 0
    for b in cuts:
        out.append((a, b))
        a = b
    return out


def _prog_agg(meta, layer):
    """L2 (layer==2): OUT = (s*M2)*relu((M1*M2)*(A*s/M1) + (M1*M2)*b1), fp8
       L4 (layer==4): OUT = A*s/M3 + b3, fp16"""
    import concourse.mybir as mybir
    import concourse.tile as tile
    f8 = mybir.dt.float8e3
    f16 = mybir.dt.float16
    f32 = mybir.dt.float32
    Alu = mybir.AluOpType
    Act = mybir.ActivationFunctionType
    NCOL, SLOTS = meta["NCOL"], meta["SLOTS"]
    nc = _mk_bass()

    MSG_d = nc.dram_tensor("MSG", [P, SLOTS], f8, kind="ExternalInput")
    SA_d = nc.dram_tensor("SA", [P, NCOL], f16, kind="ExternalInput")
    BK_d = nc.dram_tensor("BK", [P, 1], f32, kind="ExternalInput")
    ID_d = nc.dram_tensor("ID", [P, P], f8, kind="ExternalInput")
    OUT_d = nc.dram_tensor("OUT", [P, NCOL], f8 if layer == 2 else f16,
                           kind="ExternalOutput")

    with tile.TileContext(nc, num_cores=C) as tc:
        with (
            tc.tile_pool(name="const", bufs=1) as const,
            tc.tile_pool(name="ps", bufs=3, space="PSUM") as psp,
        ):
            ID_s = const.tile([P, P], f8)
            nc.sync.dma_start(ID_s[:], ID_d[:])
            SA_s = const.tile([P, NCOL], f16)
            nc.sync.dma_start(SA_s[:], SA_d[:])
            BK_s = const.tile([P, 1], f32)
            nc.sync.dma_start(BK_s[:], BK_d[:])
            MSG_s = const.tile([P, SLOTS], f8)
            for (a, b) in _msg_chunks(meta["strips"], SLOTS):
                nc.sync.dma_start(MSG_s[:, a:b], MSG_d[:, a:b])
            U_s = const.tile([P, NCOL], f16)
            H_s = const.tile([P, NCOL], f16)
            OUT_s = const.tile([P, NCOL], f8 if layer == 2 else f16)

            # PE warm-up during the first MSG DMA (pstate ramp)
            wm = psp.tile([P, RW], f32, tag="ps")
            for i in range(5):
                nc.tensor.matmul(wm[:, :P], lhsT=ID_s[:], rhs=ID_s[:],
                                 start=(i == 0), stop=(i == 4))

            stored = 0
            for ri, (r0, wr, Dr, idxs) in enumerate(meta["regions"]):
                ps = psp.tile([P, RW], f32, tag="ps")
                for j, si in enumerate(idxs):
                    (_, d, w, soff) = meta["strips"][si]
                    nc.tensor.matmul(ps[:, :w], lhsT=ID_s[:],
                                     rhs=MSG_s[:, soff : soff + w],
                                     start=(j == 0), stop=(j == Dr - 1))
                cols = slice(r0, r0 + wr)
                nc.vector.tensor_tensor(out=U_s[:, cols], in0=ps[:, :wr],
                                        in1=SA_s[:, cols], op=Alu.mult)
                if layer == 2:
                    nc.scalar.activation(out=H_s[:, cols], in_=U_s[:, cols],
                                         func=Act.Relu, bias=BK_s[:],
                                         scale=float(M1 * M2))
                    eng = nc.gpsimd if ri % 3 == 2 else nc.vector
                    eng.tensor_tensor(out=OUT_s[:, cols], in0=H_s[:, cols],
                                      in1=SA_s[:, cols], op=Alu.mult)
                else:
                    nc.scalar.activation(out=OUT_s[:, cols], in_=U_s[:, cols],
                                         func=Act.Identity, bias=BK_s[:],
                                         scale=1.0)
                if ri % 2 == 1 or r0 + wr == NCOL:
                    nc.scalar.dma_start(OUT_d[:, stored : r0 + wr],
                                        OUT_s[:, stored : r0 + wr])
                    stored = r0 + wr
    nc.compile()
    return nc


def _prog_l1(meta):
    """T1 = (s*M1) * (X @ W1), two [64, NCOL] fp8 half strips."""
    import concourse.mybir as mybir
    import concourse.tile as tile
    f8 = mybir.dt.float8e3
    f16 = mybir.dt.float16
    f32 = mybir.dt.float32
    Alu = mybir.AluOpType
    NCOL = meta["NCOL"]
    nc = _mk_bass()

    XT_d = nc.dram_tensor("XT", [IN_DIM, 2 * NCOL], f16, kind="ExternalInput")
    W1_d = nc.dram_tensor("W1", [IN_DIM, HID], f16, kind="ExternalInput")
    SRT1_d = nc.dram_tensor("SRT1", [P, NCOL], f16, kind="ExternalInput")
    T1E_d = nc.dram_tensor("T1E", [HID, NCOL], f8, kind="ExternalOutput")
    T1O_d = nc.dram_tensor("T1O", [HID, NCOL], f8, kind="ExternalOutput")

    with tile.TileContext(nc, num_cores=C) as tc:
        with (
            tc.tile_pool(name="const", bufs=1) as const,
            tc.tile_pool(name="ps", bufs=4, space="PSUM") as psp,
        ):
            W1_s = const.tile([IN_DIM, HID], f16)
            nc.sync.dma_start(W1_s[:], W1_d[:])
            SRT1_s = const.tile([P, NCOL], f16)
            nc.sync.dma_start(SRT1_s[:], SRT1_d[:])
            XT_s = const.tile([IN_DIM, 2 * NCOL], f16)
            CH = 1600
            for a in range(0, 2 * NCOL, CH):
                b = min(a + CH, 2 * NCOL)
                nc.sync.dma_start(XT_s[:, a:b], XT_d[:, a:b])
            T1 = [const.tile([HID, NCOL], f8, name=f"T1{h}", tag=f"T1{h}")
                  for h in range(2)]
            T1_d = [T1E_d, T1O_d]

            flows = []
            for half in range(2):
                for a in range(0, NCOL, RW):
                    w = min(RW, NCOL - a)
                    flows.append((half, a, w))
            for i, (half, a, w) in enumerate(flows):
                ps = psp.tile([HID, RW], f32, tag="ps")
                nc.tensor.matmul(
                    ps[:, :w], lhsT=W1_s[:],
                    rhs=XT_s[:, half * NCOL + a : half * NCOL + a + w],
                    start=True, stop=True)
                nc.vector.tensor_tensor(
                    out=T1[half][:, a : a + w], in0=ps[:, :w],
                    in1=SRT1_s[half * HID : half * HID + HID, a : a + w],
                    op=Alu.mult)
            for half in range(2):
                h2 = NCOL // 2
                nc.scalar.dma_start(T1_d[half][:, 0:h2], T1[half][:, 0:h2])
                nc.scalar.dma_start(T1_d[half][:, h2:NCOL], T1[half][:, h2:NCOL])
    nc.compile()
    return nc


def _prog_l3(meta):
    """T3 = (s*M3)*(relu((s*Agg(T2)/M2)@W2 + b2)@W3), fp8 half strips.
    W3 input is pre-scaled by M2*M3 on the host."""
    import concourse.mybir as mybir
    import concourse.tile as tile
    f8 = mybir.dt.float8e3
    f16 = mybir.dt.float16
    f32 = mybir.dt.float32
    Alu = mybir.AluOpType
    Act = mybir.ActivationFunctionType
    NCOL, SLOTS = meta["NCOL"], meta["SLOTS"]
    nc = _mk_bass()

    MSG_d = nc.dram_tensor("MSG", [P, SLOTS], f8, kind="ExternalInput")
    SA_d = nc.dram_tensor("SA", [P, NCOL], f16, kind="ExternalInput")
    W2_d = nc.dram_tensor("W2", [P, F2], f16, kind="ExternalInput")
    W3_d = nc.dram_tensor("W3", [F2, HID], f16, kind="ExternalInput")
    B2_d = nc.dram_tensor("B2", [F2, 1], f32, kind="ExternalInput")
    ID_d = nc.dram_tensor("ID", [P, P], f8, kind="ExternalInput")
    T3E_d = nc.dram_tensor("T3E", [HID, NCOL], f8, kind="ExternalOutput")
    T3O_d = nc.dram_tensor("T3O", [HID, NCOL], f8, kind="ExternalOutput")

    with tile.TileContext(nc, num_cores=C) as tc:
        with (
            tc.tile_pool(name="const", bufs=1) as const,
            tc.tile_pool(name="psA", bufs=2, space="PSUM") as psAp,
            tc.tile_pool(name="ps2", bufs=2, space="PSUM") as ps2p,
            tc.tile_pool(name="ps3", bufs=2, space="PSUM") as ps3p,
        ):
            ID_s = const.tile([P, P], f8)
            nc.sync.dma_start(ID_s[:], ID_d[:])
            SA_s = const.tile([P, NCOL], f16)
            nc.sync.dma_start(SA_s[:], SA_d[:])
            W2_s = const.tile([P, F2], f16)
            nc.sync.dma_start(W2_s[:], W2_d[:])
            W3_s = const.tile([F2, HID], f16)
            nc.sync.dma_start(W3_s[:], W3_d[:])
            B2_s = const.tile([F2, 1], f32)
            nc.sync.dma_start(B2_s[:], B2_d[:])
            MSG_s = const.tile([P, SLOTS], f8)
            for (a, b) in _msg_chunks(meta["strips"], SLOTS):
                nc.sync.dma_start(MSG_s[:, a:b], MSG_d[:, a:b])
            U_s = const.tile([P, NCOL], f16)
            H2E = const.tile([F2, NCOL], f16)
            H2O = const.tile([F2, NCOL], f16)
            T3E_s = const.tile([HID, NCOL], f8)
            T3O_s = const.tile([HID, NCOL], f8)

            # PE warm-up during the first MSG DMA
            wm = psAp.tile([P, RW], f32, tag="psA")
            for i in range(5):
                nc.tensor.matmul(wm[:, :P], lhsT=ID_s[:], rhs=ID_s[:],
                                 start=(i == 0), stop=(i == 4))

            stored = [0, 0]

            def tail(ri):
                (r0, wr, Dr, idxs) = meta["regions"][ri]
                ps = ps_of[ri]
                cols = slice(r0, r0 + wr)
                nc.vector.tensor_tensor(out=U_s[:, cols], in0=ps[:, :wr],
                                        in1=SA_s[:, cols], op=Alu.mult)
                ps2s = []
                for hb, tp in ((0, (0, 0)), (HID, (HID, 0))):
                    ps2 = ps2p.tile([F2, RW], f32, tag="ps2")
                    nc.tensor.matmul(ps2[:, :wr],
                                     lhsT=W2_s[hb : hb + HID, :],
                                     rhs=U_s[hb : hb + HID, cols],
                                     start=True, stop=True, tile_position=tp)
                    ps2s.append(ps2)
                H2s = (H2E, H2O)
                for ps2, H2 in zip(ps2s, H2s):
                    nc.scalar.activation(out=H2[:, cols], in_=ps2[:, :wr],
                                         func=Act.Relu, bias=B2_s[:],
                                         scale=1.0)
                ps3 = ps3p.tile([P, RW], f32, tag="ps3")
                for hb, H2 in ((0, H2E), (HID, H2O)):
                    nc.tensor.matmul(ps3[hb : hb + HID, :wr], lhsT=W3_s[:],
                                     rhs=H2[:, cols], start=True, stop=True)
                for hb, T3_s in ((0, T3E_s), (HID, T3O_s)):
                    nc.vector.tensor_tensor(
                        out=T3_s[:, cols], in0=ps3[hb : hb + HID, :wr],
                        in1=SA_s[hb : hb + HID, cols], op=Alu.mult)
                if ri % 2 == 1 or r0 + wr == NCOL:
                    for h, (T3_s, T3_d) in enumerate(
                            ((T3E_s, T3E_d), (T3O_s, T3O_d))):
                        nc.scalar.dma_start(T3_d[:, stored[h] : r0 + wr],
                                            T3_s[:, stored[h] : r0 + wr])
                        stored[h] = r0 + wr

            ps_of = {}
            for ri, (r0, wr, Dr, idxs) in enumerate(meta["regions"]):
                ps = psAp.tile([P, RW], f32, tag="psA")
                ps_of[ri] = ps
                for j, si in enumerate(idxs):
                    (_, d, w, soff) = meta["strips"][si]
                    nc.tensor.matmul(ps[:, :w], lhsT=ID_s[:],
                                     rhs=MSG_s[:, soff : soff + w],
                                     start=(j == 0), stop=(j == Dr - 1))
                if ri >= 1:
                    tail(ri - 1)
            tail(len(meta["regions"]) - 1)
    nc.compile()
    return nc


def _prog_pool(meta):
    """Per-graph max over depth-major [128, S2*GPC] fp16; OUT [GPC, HID] f32."""
    import concourse.mybir as mybir
    import concourse.tile as tile
    from concourse.masks import make_identity
    f16 = mybir.dt.float16
    f32 = mybir.dt.float32
    Alu = mybir.AluOpType
    S2 = meta["S2"]
    nc = _mk_bass()

    MSGP_d = nc.dram_tensor("MSGP", [P, S2 * GPC], f16, kind="ExternalInput")
    OUT_d = nc.dram_tensor("OUT", [GPC, HID], f32, kind="ExternalOutput")

    with tile.TileContext(nc, num_cores=C) as tc:
        with (
            tc.tile_pool(name="const", bufs=1) as const,
            tc.tile_pool(name="ps", bufs=2, space="PSUM") as psp,
        ):
            mg = const.tile([P, S2 * GPC], f16)
            nch = 4
            step = -(-S2 // nch) * GPC
            for a in range(0, S2 * GPC, step):
                b = min(a + step, S2 * GPC)
                nc.sync.dma_start(mg[:, a:b], MSGP_d[:, a:b])
            v = mg.rearrange("p (d g) -> p d g", g=GPC)
            cur = S2
            while cur > 1:
                h = cur // 2
                if cur % 2:
                    nc.vector.tensor_tensor(
                        out=v[:, 0, :], in0=v[:, 0, :],
                        in1=v[:, cur - 1, :], op=Alu.max)
                nc.vector.tensor_tensor(
                    out=v[:, 0:h, :], in0=v[:, 0:h, :],
                    in1=v[:, h : 2 * h, :], op=Alu.max)
                cur = h
            ident = const.tile([P, P], f16)
            make_identity(nc, ident[:])
            psT = psp.tile([GPC, P], f16, tag="t")
            nc.tensor.transpose(psT[:], mg[:, 0:GPC], ident[:])
            sT = const.tile([GPC, P], f16)
            nc.vector.tensor_copy(sT[:], psT[:])
            OUT_s = const.tile([GPC, HID], f32)
            nc.vector.tensor_tensor(out=OUT_s[:], in0=sT[:, 0:HID],
                                    in1=sT[:, HID:P], op=Alu.max)
            nc.sync.dma_start(OUT_d[:], OUT_s[:])
    nc.compile()
    return nc


# --------------------------------------------------------------------------
# Entry point
# --------------------------------------------------------------------------

_RUN_KWARGS = {}
_EXEC_NS = []
_PROFILE = False


def _stage_msgs(T_full, srcmap_c):
    """[N+1, HID] table + [2, SLOTS] slot->row map -> [128, SLOTS]."""
    top = T_full[srcmap_c[0]].T      # [64, SLOTS]
    bot = T_full[srcmap_c[1]].T
    return np.ascontiguousarray(np.concatenate([top, bot], axis=0))


def _assemble(prep, parts_E, parts_O, dtype):
    """Per-core [64, NCOL] half strips -> full [N+1, HID] table."""
    T_full = np.zeros((N + 1, HID), dtype)
    for c in range(C):
        tops, bots = prep["tops"][c], prep["bots"][c]
        mE, mO = tops >= 0, bots >= 0
        T_full[tops[mE]] = parts_E[c][:, mE].T
        T_full[bots[mO]] = parts_O[c][:, mO].T
    return T_full


def kernel(data, edge_index, batch, W1, b1, W2, b2, W3, b3):
    from concourse.bass_utils import run_bass_kernel_spmd

    data = np.asarray(data, dtype=np.float32)
    edge_index = np.asarray(edge_index, dtype=np.int32)
    batch_np = np.asarray(batch, dtype=np.int32)

    prep = _host_prep(edge_index, batch_np)
    meta = prep["meta"]
    NCOL = meta["NCOL"]

    IDENT8 = np.eye(P, dtype=F8)
    W1f = np.asarray(W1, np.float32).astype(F16)            # [128, 64]
    W2f = np.asarray(W2, np.float32).astype(F16)            # [64, 128]
    W3f = (np.asarray(W3, np.float32) * (M2 * M3)).astype(F16)
    B1K = (np.tile(np.asarray(b1, np.float32), 2) * (M1 * M2))[:, None].copy()
    B2r = np.asarray(b2, np.float32)[:, None].copy()
    B3r = np.tile(np.asarray(b3, np.float32), 2)[:, None].copy()

    Xx = np.concatenate([data, np.zeros((1, IN_DIM), np.float32)], axis=0)
    XT = np.empty((C, IN_DIM, 2 * NCOL), F16)
    for c in range(C):
        XT[c, :, :NCOL] = Xx[prep["topsx"][c]].T
        XT[c, :, NCOL:] = Xx[prep["botsx"][c]].T

    cores = list(range(C))
    del _EXEC_NS[:]

    def run(nc, in_maps):
        if _PROFILE:
            from concourse.timeline_sim import TimelineSim
            _EXEC_NS.append(TimelineSim(nc, require_finite=False).simulate())
        res = run_bass_kernel_spmd(nc, in_maps, cores, **_RUN_KWARGS)
        if res.exec_time_ns is not None:
            _EXEC_NS.append(res.exec_time_ns)
        return res.results

    # ---- L1: T1 = (s*M1) * (X @ W1) ----
    r1 = run(_prog_l1(meta),
             [{"XT": np.ascontiguousarray(XT[c]), "W1": W1f,
               "SRT1": np.ascontiguousarray(prep["SRT1"][c])}
              for c in range(C)])
    T1 = _assemble(prep,
                   [np.asarray(r1[c]["T1E"]) for c in range(C)],
                   [np.asarray(r1[c]["T1O"]) for c in range(C)], F8)

    # ---- L2: T2 = (s*M2)*relu(s*Agg(T1)/M1 + b1) ----
    r2 = run(_prog_agg(meta, 2),
             [{"MSG": _stage_msgs(T1, prep["srcmap"][c]),
               "SA": np.ascontiguousarray(prep["SA2"][c]), "BK": B1K,
               "ID": IDENT8}
              for c in range(C)])
    T2 = _assemble(prep,
                   [np.asarray(r2[c]["OUT"])[0:HID] for c in range(C)],
                   [np.asarray(r2[c]["OUT"])[HID:P] for c in range(C)], F8)

    # ---- L3: T3 = (s*M3)*(relu((s*Agg(T2)/M2)@W2 + b2)@W3) ----
    r3 = run(_prog_l3(meta),
             [{"MSG": _stage_msgs(T2, prep["srcmap"][c]),
               "SA": np.ascontiguousarray(prep["SA3"][c]),
               "W2": np.concatenate([W2f, W2f], axis=0), "W3": W3f,
               "B2": B2r, "ID": IDENT8}
              for c in range(C)])
    T3 = _assemble(prep,
                   [np.asarray(r3[c]["T3E"]) for c in range(C)],
                   [np.asarray(r3[c]["T3O"]) for c in range(C)], F8)

    # ---- L4: H3 = s*Agg(T3)/M3 + b3 ----
    r4 = run(_prog_agg(meta, 4),
             [{"MSG": _stage_msgs(T3, prep["srcmap"][c]),
               "SA": np.ascontiguousarray(prep["SA4"][c]), "BK": B3r,
               "ID": IDENT8}
              for c in range(C)])
    H3 = _assemble(prep,
                   [np.asarray(r4[c]["OUT"])[0:HID] for c in range(C)],
                   [np.asarray(r4[c]["OUT"])[HID:P] for c in range(C)], F16)
    H3[N] = np.float16(-60000.0)     # pad row for the pool staging

    # ---- L5: per-graph max pool ----
    r5 = run(_prog_pool(meta),
             [{"MSGP": _stage_msgs(H3, prep["poolmap"][c])}
              for c in range(C)])
    out = np.concatenate([np.asarray(r5[c]["OUT"]) for c in range(C)],
                         axis=0).astype(np.float32)
    out[prep["cnt"].reshape(-1) == 0] = -np.inf
    return out


# revision 17
# speedup vs baseline: 1.4111x; 1.0411x over previous
"""Trainium2 Bass kernel for a 3-layer GCN encoder with global max pool.

Strategy (8 NeuronCores, SPMD, 5 launches, host staging between launches):
  - Nodes partitioned graph-wise (graph g -> core g//64). The host only MOVES
    device-computed bytes between launches (gather rows into padded message
    tables); every FLOP runs on device.
  - Aggregation layers stage per-edge messages in fp8 (e3m4) with fixed
    power-free scale factors folded into the device-side s-tables, halving
    HBM traffic vs fp16.
  - The aggregation sum runs on the Tensor engine: identity-weight matmuls
    accumulate message strips into PSUM (start/stop prefix accumulation).
    Columns (node pairs) are sorted by descending in-degree so the set of
    columns with a message at depth d is a prefix; strips are stored
    region-major (one 512-column PSUM bank region at a time) so PSUM holds
    each region until its sum completes.
  - Post-ops per region: DVE multiplies PSUM by the s-table; Activation
    applies (scaled) bias+relu; DVE writes the next layer's pre-scaled
    fp8 table directly.
  - Launches: L1  T1 = (s*M1) * (X @ W1)                 [fp8 out]
              L2  T2 = (s*M2) * relu(s*Agg(T1)/M1 + b1)  [fp8 out]
              L3  T3 = (s*M3) * (relu((s*Agg(T2)/M2)@W2 + b2)@W3)
              L4  H3 = s*Agg(T3)/M3 + b3                 [fp16 out]
              L5  per-graph max pool over H3 (depth-major staged layout)
"""

import numpy as np
import ml_dtypes

N = 50000
IN_DIM = 128
HID = 64
F2 = 2 * HID
N_GRAPHS = 512
C = 8
P = 128
GPC = N_GRAPHS // C
RW = 512            # psum region width (columns)
F16 = np.float16
F8 = ml_dtypes.float8_e3m4

M1, M2, M3 = 5.0, 11.0, 44.0   # staging scale factors (fold into s-tables)

# L3 aggregation offload: region -> (PE strip count, engine for the rest)
_L3_OFF = {}


# --------------------------------------------------------------------------
# Host-side preprocessing (graph structure only - no feature arithmetic)
# --------------------------------------------------------------------------

def _host_prep(edge_index, batch):
    src = np.asarray(edge_index[0], dtype=np.int64)
    dst = np.asarray(edge_index[1], dtype=np.int64)
    batch = np.asarray(batch, dtype=np.int64)
    core_of = batch // GPC

    indeg = np.bincount(dst, minlength=N)
    k = indeg + 1                     # slots per node incl. self loop
    s = (1.0 / np.sqrt(k.astype(np.float64))).astype(np.float32)

    # in-neighbor lists grouped by dst
    eorder = np.argsort(dst, kind="stable")
    esrc = src[eorder]
    estart = np.zeros(N + 1, np.int64)
    np.cumsum(np.bincount(dst, minlength=N), out=estart[1:])

    # per-core node order: descending k, paired (2i, 2i+1) into columns
    orders = []
    for c in range(C):
        nodes = np.nonzero(core_of == c)[0]
        orders.append(nodes[np.argsort(-k[nodes], kind="stable")])
    NCOL = max((len(o) + 1) // 2 for o in orders)

    tops = np.full((C, NCOL), -1, np.int64)
    bots = np.full((C, NCOL), -1, np.int64)
    for c in range(C):
        o = orders[c]
        tops[c, : len(o[0::2])] = o[0::2]
        bots[c, : len(o[1::2])] = o[1::2]
    topsx = np.where(tops >= 0, tops, N)
    botsx = np.where(bots >= 0, bots, N)

    # column depth = max over cores of max(k_top, k_bot); >=1 (self loop)
    kk = np.concatenate([k, [0]])
    D_col = np.maximum(kk[topsx], kk[botsx]).max(axis=0)
    D_col = np.maximum(D_col, 1)
    assert (np.diff(D_col) <= 0).all()
    DMAX = int(D_col[0])
    n_d = np.array([(D_col > d).sum() for d in range(DMAX)], np.int64)

    # region-major strips: (region_col0, d, w, slot_off)
    strips = []
    off = 0
    regions = []          # (col0, width, D_r, [strip indices])
    for r0 in range(0, NCOL, RW):
        wr = min(RW, NCOL - r0)
        Dr = int(D_col[r0])
        idxs = []
        for d in range(Dr):
            w = int(min(n_d[d] - r0, wr))
            assert w > 0
            idxs.append(len(strips))
            strips.append((r0, d, w, off))
            off += w
        regions.append((r0, wr, Dr, idxs))
    SLOTS = off

    # slot -> source node maps (N = zero row) for tops/bottoms
    indegx = np.concatenate([indeg, [0]])
    estartx = np.concatenate([estart[:-1], [0]])
    srcmap = np.full((C, 2, SLOTS), N, np.int64)
    for (r0, d, w, soff) in strips:
        for c in range(C):
            for half, nodes_h in ((0, topsx[c]), (1, botsx[c])):
                v = nodes_h[r0 : r0 + w]
                if d == 0:
                    srcmap[c, half, soff : soff + w] = v
                else:
                    sel = (d <= indegx[v]) & (v < N)
                    tgt = srcmap[c, half, soff : soff + w]
                    tgt[sel] = esrc[estartx[v[sel]] + d - 1]

    # s tables [C, 128, NCOL] fp16: top s in rows 0:64, bot s in rows 64:128
    def s_table(scale):
        sx = np.concatenate([s * scale, [0.0]]).astype(F16)
        top = sx[topsx][:, None, :].repeat(HID, axis=1)
        bot = sx[botsx][:, None, :].repeat(HID, axis=1)
        return np.concatenate([top, bot], axis=1)      # [C, 128, NCOL]

    SRT1 = s_table(M1)
    SA2 = s_table(1.0 / M1)
    SA3 = s_table(1.0 / M2)
    SA4 = s_table(1.0 / M3)

    # pooling: two graph-groups, depth-major within each: for group G the
    # column at G*(S2*GH) + d*GH + j holds the d-th node pair of graph
    # G*GH + j  (GH = GPC//2 graphs per group)
    GH = GPC // 2
    gl = batch % GPC
    cnt = np.zeros((C, GPC), np.int64)
    np.add.at(cnt, (core_of, gl), 1)
    S2 = int(-(-cnt.max() // 2))
    poolmap = np.full((C, 2, 2 * S2 * GH), N, np.int64)
    for c in range(C):
        for g in range(GPC):
            nodes = np.nonzero((core_of == c) & (gl == g))[0]
            e = nodes[0::2]
            o = nodes[1::2]
            base = (g // GH) * (S2 * GH) + (g % GH)
            poolmap[c, 0, base : base + S2 * GH : GH][: len(e)] = e
            poolmap[c, 1, base : base + S2 * GH : GH][: len(o)] = o

    meta = dict(NCOL=NCOL, SLOTS=SLOTS, S2=S2, strips=strips,
                regions=regions)
    return dict(meta=meta, tops=tops, bots=bots, topsx=topsx, botsx=botsx,
                srcmap=srcmap, poolmap=poolmap, cnt=cnt,
                SRT1=SRT1, SA2=SA2, SA3=SA3, SA4=SA4)


# --------------------------------------------------------------------------
# Bass programs
# --------------------------------------------------------------------------

def _mk_bass():
    import concourse.bacc as bacc
    return bacc.Bacc(None)


def _msg_chunks(strips, SLOTS, first=1400, later=3600):
    """Split the slot axis into DMA chunks at strip boundaries."""
    cuts = []
    target = first
    for (r0, d, w, soff) in strips:
        end = soff + w
        if end >= target:
            cuts.append(end)
            target = end + later
    if not cuts or cuts[-1] != SLOTS:
        cuts.append(SLOTS)
    out = []
    a = 0
    for b in cuts:
        out.append((a, b))
        a = b
    return out


def _prog_agg(meta, layer):
    """L2 (layer==2): OUT = (s*M2)*relu((M1*M2)*(A*s/M1) + (M1*M2)*b1), fp8
       L4 (layer==4): OUT = A*s/M3 + b3, fp16"""
    import concourse.mybir as mybir
    import concourse.tile as tile
    f8 = mybir.dt.float8e3
    f16 = mybir.dt.float16
    f32 = mybir.dt.float32
    Alu = mybir.AluOpType
    Act = mybir.ActivationFunctionType
    NCOL, SLOTS = meta["NCOL"], meta["SLOTS"]
    nc = _mk_bass()

    MSG_d = nc.dram_tensor("MSG", [P, SLOTS], f8, kind="ExternalInput")
    SA_d = nc.dram_tensor("SA", [P, NCOL], f16, kind="ExternalInput")
    BK_d = nc.dram_tensor("BK", [P, 1], f32, kind="ExternalInput")
    ID_d = nc.dram_tensor("ID", [P, P], f8, kind="ExternalInput")
    OUT_d = nc.dram_tensor("OUT", [P, NCOL], f8 if layer == 2 else f16,
                           kind="ExternalOutput")

    with tile.TileContext(nc, num_cores=C) as tc:
        with (
            tc.tile_pool(name="const", bufs=1) as const,
            tc.tile_pool(name="ps", bufs=3, space="PSUM") as psp,
        ):
            ID_s = const.tile([P, P], f8)
            nc.sync.dma_start(ID_s[:], ID_d[:])
            MSG_s = const.tile([P, SLOTS], f8)
            chunks = _msg_chunks(meta["strips"], SLOTS)
            SA_s = const.tile([P, NCOL], f16)
            BK_s = const.tile([P, 1], f32)
            for ci, (a, b) in enumerate(chunks):
                nc.sync.dma_start(MSG_s[:, a:b], MSG_d[:, a:b])
                if ci == 1:
                    nc.sync.dma_start(SA_s[:], SA_d[:])
                    nc.sync.dma_start(BK_s[:], BK_d[:])
            U_s = const.tile([P, NCOL], f16)
            H_s = const.tile([P, NCOL], f16)
            OUT_s = const.tile([P, NCOL], f8 if layer == 2 else f16)

            # PE warm-up during the first MSG DMA (pstate ramp)
            wm = psp.tile([P, RW], f32, tag="ps")
            for i in range(8):
                nc.tensor.matmul(wm[:, :P], lhsT=ID_s[:], rhs=ID_s[:],
                                 start=(i == 0), stop=(i == 7))

            stored = 0
            for ri, (r0, wr, Dr, idxs) in enumerate(meta["regions"]):
                ps = psp.tile([P, RW], f32, tag="ps")
                for j, si in enumerate(idxs):
                    (_, d, w, soff) = meta["strips"][si]
                    nc.tensor.matmul(ps[:, :w], lhsT=ID_s[:],
                                     rhs=MSG_s[:, soff : soff + w],
                                     start=(j == 0), stop=(j == Dr - 1))
                cols = slice(r0, r0 + wr)
                nc.vector.tensor_tensor(out=U_s[:, cols], in0=ps[:, :wr],
                                        in1=SA_s[:, cols], op=Alu.mult)
                if layer == 2:
                    nc.scalar.activation(out=H_s[:, cols], in_=U_s[:, cols],
                                         func=Act.Relu, bias=BK_s[:],
                                         scale=float(M1 * M2))
                    eng = nc.gpsimd if ri in (1, 3) else nc.vector
                    eng.tensor_tensor(out=OUT_s[:, cols], in0=H_s[:, cols],
                                      in1=SA_s[:, cols], op=Alu.mult)
                else:
                    nc.scalar.activation(out=OUT_s[:, cols], in_=U_s[:, cols],
                                         func=Act.Identity, bias=BK_s[:],
                                         scale=1.0)
                if ri % 2 == 1 or r0 + wr == NCOL:
                    nc.scalar.dma_start(OUT_d[:, stored : r0 + wr],
                                        OUT_s[:, stored : r0 + wr])
                    stored = r0 + wr
    nc.compile()
    return nc


def _prog_l1(meta):
    """T1 = (s*M1) * (X @ W1), two [64, NCOL] fp8 half strips."""
    import concourse.mybir as mybir
    import concourse.tile as tile
    f8 = mybir.dt.float8e3
    f16 = mybir.dt.float16
    f32 = mybir.dt.float32
    Alu = mybir.AluOpType
    NCOL = meta["NCOL"]
    nc = _mk_bass()

    XT_d = nc.dram_tensor("XT", [IN_DIM, 2 * NCOL], f16, kind="ExternalInput")
    W1_d = nc.dram_tensor("W1", [IN_DIM, HID], f16, kind="ExternalInput")
    SRT1_d = nc.dram_tensor("SRT1", [P, NCOL], f16, kind="ExternalInput")
    T1P_d = nc.dram_tensor("T1P", [P, NCOL], f8, kind="ExternalOutput")

    with tile.TileContext(nc, num_cores=C) as tc:
        with (
            tc.tile_pool(name="const", bufs=1) as const,
            tc.tile_pool(name="ps", bufs=4, space="PSUM") as psp,
        ):
            W1_s = const.tile([IN_DIM, HID], f16)
            nc.sync.dma_start(W1_s[:], W1_d[:])
            XT_s = const.tile([IN_DIM, 2 * NCOL], f16)
            SRT1_s = const.tile([P, NCOL], f16)
            # top-strip chunk0 and bot-strip chunk0 first so the first
            # packed column-pair flows start early; SRT1 deferred
            cuts = [RW, 2 * RW] + list(range(4 * RW, NCOL, 4 * RW)) + [NCOL]
            chunks = []
            a = 0
            for b in cuts:
                if b > a:
                    chunks.append((a, b))
                    a = b
            for ci, (a, b) in enumerate(chunks):
                nc.sync.dma_start(XT_s[:, a:b], XT_d[:, a:b])
                nc.sync.dma_start(XT_s[:, NCOL + a : NCOL + b],
                                  XT_d[:, NCOL + a : NCOL + b])
                if ci == 0:
                    nc.sync.dma_start(SRT1_s[:], SRT1_d[:])
            T1_s = const.tile([P, NCOL], f8)

            # brief PE warm-up while chunk0 loads
            wm = psp.tile([P, RW], f32, tag="ps")
            for i in range(4):
                nc.tensor.matmul(wm[0:HID, :HID], lhsT=W1_s[:], rhs=W1_s[:],
                                 start=(i == 0), stop=(i == 3))

            stored = 0
            nflows = -(-NCOL // RW)
            for i in range(nflows):
                a = i * RW
                w = min(RW, NCOL - a)
                ps = psp.tile([P, RW], f32, tag="ps")
                for half in range(2):
                    nc.tensor.matmul(
                        ps[half * HID : half * HID + HID, :w], lhsT=W1_s[:],
                        rhs=XT_s[:, half * NCOL + a : half * NCOL + a + w],
                        start=True, stop=True)
                nc.vector.tensor_tensor(
                    out=T1_s[:, a : a + w], in0=ps[:, :w],
                    in1=SRT1_s[:, a : a + w], op=Alu.mult)
                if i % 2 == 1 or a + w == NCOL:
                    nc.scalar.dma_start(T1P_d[:, stored : a + w],
                                        T1_s[:, stored : a + w])
                    stored = a + w
    nc.compile()
    return nc


def _prog_l3(meta):
    """T3 = (s*M3)*(relu((s*Agg(T2)/M2)@W2 + b2)@W3), fp8 half strips.
    W3 input is pre-scaled by M2*M3 on the host."""
    import concourse.mybir as mybir
    import concourse.tile as tile
    f8 = mybir.dt.float8e3
    f16 = mybir.dt.float16
    f32 = mybir.dt.float32
    Alu = mybir.AluOpType
    Act = mybir.ActivationFunctionType
    NCOL, SLOTS = meta["NCOL"], meta["SLOTS"]
    nc = _mk_bass()

    MSG_d = nc.dram_tensor("MSG", [P, SLOTS], f8, kind="ExternalInput")
    SA_d = nc.dram_tensor("SA", [P, NCOL], f16, kind="ExternalInput")
    W2_d = nc.dram_tensor("W2", [P, F2], f16, kind="ExternalInput")
    W3_d = nc.dram_tensor("W3", [F2, HID], f16, kind="ExternalInput")
    B2_d = nc.dram_tensor("B2", [F2, 1], f32, kind="ExternalInput")
    ID_d = nc.dram_tensor("ID", [P, P], f8, kind="ExternalInput")
    T3P_d = nc.dram_tensor("T3P", [P, NCOL], f8, kind="ExternalOutput")

    # deep strips of these regions are tree-summed on DVE/Pool into an fp16
    # partial; PE folds the partial into PSUM with one extra pass
    OFF = dict(_L3_OFF)

    with tile.TileContext(nc, num_cores=C) as tc:
        with (
            tc.tile_pool(name="const", bufs=1) as const,
            tc.tile_pool(name="psA", bufs=2, space="PSUM") as psAp,
            tc.tile_pool(name="ps2", bufs=2, space="PSUM") as ps2p,
            tc.tile_pool(name="ps3", bufs=2, space="PSUM") as ps3p,
        ):
            ID_s = const.tile([P, P], f8)
            nc.sync.dma_start(ID_s[:], ID_d[:])
            MSG_s = const.tile([P, SLOTS], f8)
            chunks = _msg_chunks(meta["strips"], SLOTS)
            SA_s = const.tile([P, NCOL], f16)
            W2_s = const.tile([P, F2], f16)
            W3_s = const.tile([F2, HID], f16)
            B2_s = const.tile([F2, 1], f32)
            for ci, (a, b) in enumerate(chunks):
                nc.sync.dma_start(MSG_s[:, a:b], MSG_d[:, a:b])
                if ci == 0:
                    nc.sync.dma_start(W2_s[:], W2_d[:])
                    nc.sync.dma_start(W3_s[:], W3_d[:])
                    nc.sync.dma_start(B2_s[:], B2_d[:])
                elif ci == 1:
                    nc.sync.dma_start(SA_s[:], SA_d[:])
            U_s = const.tile([P, NCOL], f16)
            H2E = const.tile([F2, NCOL], f16)
            H2O = const.tile([F2, NCOL], f16)
            T3_s = const.tile([P, NCOL], f8)
            ACC = const.tile([P, NCOL], f16)

            # PE warm-up during the first MSG DMA
            wm = psAp.tile([P, RW], f32, tag="psA")
            for i in range(8):
                nc.tensor.matmul(wm[:, :P], lhsT=ID_s[:], rhs=ID_s[:],
                                 start=(i == 0), stop=(i == 7))

            stored = [0]

            def tail(ri):
                (r0, wr, Dr, idxs) = meta["regions"][ri]
                ps = ps_of[ri]
                cols = slice(r0, r0 + wr)
                nc.vector.tensor_tensor(out=U_s[:, cols], in0=ps[:, :wr],
                                        in1=SA_s[:, cols], op=Alu.mult)
                ps2s = []
                for hb, tp in ((0, (0, 0)), (HID, (HID, 0))):
                    ps2 = ps2p.tile([F2, RW], f32, tag="ps2")
                    nc.tensor.matmul(ps2[:, :wr],
                                     lhsT=W2_s[hb : hb + HID, :],
                                     rhs=U_s[hb : hb + HID, cols],
                                     start=True, stop=True, tile_position=tp)
                    ps2s.append(ps2)
                H2s = (H2E, H2O)
                for ps2, H2 in zip(ps2s, H2s):
                    nc.scalar.activation(out=H2[:, cols], in_=ps2[:, :wr],
                                         func=Act.Relu, bias=B2_s[:],
                                         scale=1.0)
                ps3 = ps3p.tile([P, RW], f32, tag="ps3")
                for hb, H2 in ((0, H2E), (HID, H2O)):
                    nc.tensor.matmul(ps3[hb : hb + HID, :wr], lhsT=W3_s[:],
                                     rhs=H2[:, cols], start=True, stop=True)
                nc.vector.tensor_tensor(out=T3_s[:, cols], in0=ps3[:, :wr],
                                        in1=SA_s[:, cols], op=Alu.mult)
                if ri % 2 == 1 or r0 + wr == NCOL:
                    nc.scalar.dma_start(T3P_d[:, stored[0] : r0 + wr],
                                        T3_s[:, stored[0] : r0 + wr])
                    stored[0] = r0 + wr

            ps_of = {}
            for ri, (r0, wr, Dr, idxs) in enumerate(meta["regions"]):
                split, eng_name = OFF.get(ri, (Dr, None))
                split = min(split, Dr)
                if split < Dr:
                    eng = nc.vector if eng_name == "dve" else nc.gpsimd
                    first = True
                    for si in idxs[split:]:
                        (_, d, w, soff) = meta["strips"][si]
                        strip = MSG_s[:, soff : soff + w]
                        if first:
                            eng.tensor_copy(ACC[:, r0 : r0 + w], strip)
                            first = False
                        else:
                            eng.tensor_tensor(out=ACC[:, r0 : r0 + w],
                                              in0=ACC[:, r0 : r0 + w],
                                              in1=strip, op=Alu.add)
                ps = psAp.tile([P, RW], f32, tag="psA")
                ps_of[ri] = ps
                for j, si in enumerate(idxs[:split]):
                    (_, d, w, soff) = meta["strips"][si]
                    nc.tensor.matmul(ps[:, :w], lhsT=ID_s[:],
                                     rhs=MSG_s[:, soff : soff + w],
                                     start=(j == 0),
                                     stop=(j == Dr - 1 and split == Dr))
                if split < Dr:
                    (_, d, w, soff) = meta["strips"][idxs[split]]
                    nc.tensor.matmul(ps[:, :w], lhsT=ID_s[:],
                                     rhs=ACC[:, r0 : r0 + w],
                                     start=False, stop=True)
                if ri >= 1:
                    tail(ri - 1)
            tail(len(meta["regions"]) - 1)
    nc.compile()
    return nc


def _prog_pool(meta):
    """Per-graph max over two depth-major groups [128, 2*(S2*GH)] fp16;
    OUTF [HID, GPC] f32 is features x graphs (host transposes)."""
    import concourse.mybir as mybir
    import concourse.tile as tile
    f16 = mybir.dt.float16
    f32 = mybir.dt.float32
    Alu = mybir.AluOpType
    S2 = meta["S2"]
    GH = GPC // 2
    nc = _mk_bass()

    MSGP_d = nc.dram_tensor("MSGP", [P, 2 * S2 * GH], f16,
                            kind="ExternalInput")
    OUTF_d = nc.dram_tensor("OUTF", [HID, GPC], f32, kind="ExternalOutput")

    with tile.TileContext(nc, num_cores=C) as tc:
        with (
            tc.tile_pool(name="const", bufs=1) as const,
            tc.tile_pool(name="ps", bufs=2, space="PSUM") as psp,
        ):
            mg = const.tile([P, 2 * S2 * GH], f16)
            OUTF_s = const.tile([HID, GPC], f32)
            for grp in range(2):
                g0 = grp * S2 * GH
                nc.sync.dma_start(mg[:, g0 : g0 + S2 * GH],
                                  MSGP_d[:, g0 : g0 + S2 * GH])
            for grp in range(2):
                g0 = grp * S2 * GH
                v = mg[:, g0 : g0 + S2 * GH].rearrange(
                    "p (d g) -> p d g", g=GH)
                cur = S2
                while cur > 1:
                    h = cur // 2
                    if cur % 2:
                        nc.vector.tensor_tensor(
                            out=v[:, 0, :], in0=v[:, 0, :],
                            in1=v[:, cur - 1, :], op=Alu.max)
                    nc.vector.tensor_tensor(
                        out=v[:, 0:h, :], in0=v[:, 0:h, :],
                        in1=v[:, h : 2 * h, :], op=Alu.max)
                    cur = h
                bot = psp.tile([HID, GH], f32, tag="bot")
                nc.vector.tensor_copy(bot[:], mg[HID:P, g0 : g0 + GH])
                nc.vector.tensor_tensor(
                    out=OUTF_s[:, grp * GH : (grp + 1) * GH],
                    in0=mg[0:HID, g0 : g0 + GH],
                    in1=bot[:], op=Alu.max)
            nc.sync.dma_start(OUTF_d[:], OUTF_s[:])
    nc.compile()
    return nc


# --------------------------------------------------------------------------
# Entry point
# --------------------------------------------------------------------------

_RUN_KWARGS = {}
_EXEC_NS = []
_PROFILE = False


def _stage_msgs(T_full, srcmap_c):
    """[N+1, HID] table + [2, SLOTS] slot->row map -> [128, SLOTS]."""
    top = T_full[srcmap_c[0]].T      # [64, SLOTS]
    bot = T_full[srcmap_c[1]].T
    return np.ascontiguousarray(np.concatenate([top, bot], axis=0))


def _assemble(prep, parts_E, parts_O, dtype):
    """Per-core [64, NCOL] half strips -> full [N+1, HID] table."""
    T_full = np.zeros((N + 1, HID), dtype)
    for c in range(C):
        tops, bots = prep["tops"][c], prep["bots"][c]
        mE, mO = tops >= 0, bots >= 0
        T_full[tops[mE]] = parts_E[c][:, mE].T
        T_full[bots[mO]] = parts_O[c][:, mO].T
    return T_full


def kernel(data, edge_index, batch, W1, b1, W2, b2, W3, b3):
    from concourse.bass_utils import run_bass_kernel_spmd

    data = np.asarray(data, dtype=np.float32)
    edge_index = np.asarray(edge_index, dtype=np.int32)
    batch_np = np.asarray(batch, dtype=np.int32)

    prep = _host_prep(edge_index, batch_np)
    meta = prep["meta"]
    NCOL = meta["NCOL"]

    IDENT8 = np.eye(P, dtype=F8)
    W1f = np.asarray(W1, np.float32).astype(F16)            # [128, 64]
    W2f = np.asarray(W2, np.float32).astype(F16)            # [64, 128]
    W3f = (np.asarray(W3, np.float32) * (M2 * M3)).astype(F16)
    B1K = (np.tile(np.asarray(b1, np.float32), 2) * (M1 * M2))[:, None].copy()
    B2r = np.asarray(b2, np.float32)[:, None].copy()
    B3r = np.tile(np.asarray(b3, np.float32), 2)[:, None].copy()

    Xx = np.concatenate([data, np.zeros((1, IN_DIM), np.float32)], axis=0)
    XT = np.empty((C, IN_DIM, 2 * NCOL), F16)
    for c in range(C):
        XT[c, :, :NCOL] = Xx[prep["topsx"][c]].T
        XT[c, :, NCOL:] = Xx[prep["botsx"][c]].T

    cores = list(range(C))
    del _EXEC_NS[:]

    def run(nc, in_maps):
        if _PROFILE:
            from concourse.timeline_sim import TimelineSim
            _EXEC_NS.append(TimelineSim(nc, require_finite=False).simulate())
        res = run_bass_kernel_spmd(nc, in_maps, cores, **_RUN_KWARGS)
        if res.exec_time_ns is not None:
            _EXEC_NS.append(res.exec_time_ns)
        return res.results

    # ---- L1: T1 = (s*M1) * (X @ W1) ----
    r1 = run(_prog_l1(meta),
             [{"XT": np.ascontiguousarray(XT[c]), "W1": W1f,
               "SRT1": np.ascontiguousarray(prep["SRT1"][c])}
              for c in range(C)])
    T1 = _assemble(prep,
                   [np.asarray(r1[c]["T1P"])[0:HID] for c in range(C)],
                   [np.asarray(r1[c]["T1P"])[HID:P] for c in range(C)], F8)

    # ---- L2: T2 = (s*M2)*relu(s*Agg(T1)/M1 + b1) ----
    r2 = run(_prog_agg(meta, 2),
             [{"MSG": _stage_msgs(T1, prep["srcmap"][c]),
               "SA": np.ascontiguousarray(prep["SA2"][c]), "BK": B1K,
               "ID": IDENT8}
              for c in range(C)])
    T2 = _assemble(prep,
                   [np.asarray(r2[c]["OUT"])[0:HID] for c in range(C)],
                   [np.asarray(r2[c]["OUT"])[HID:P] for c in range(C)], F8)

    # ---- L3: T3 = (s*M3)*(relu((s*Agg(T2)/M2)@W2 + b2)@W3) ----
    r3 = run(_prog_l3(meta),
             [{"MSG": _stage_msgs(T2, prep["srcmap"][c]),
               "SA": np.ascontiguousarray(prep["SA3"][c]),
               "W2": np.concatenate([W2f, W2f], axis=0), "W3": W3f,
               "B2": B2r, "ID": IDENT8}
              for c in range(C)])
    T3 = _assemble(prep,
                   [np.asarray(r3[c]["T3P"])[0:HID] for c in range(C)],
                   [np.asarray(r3[c]["T3P"])[HID:P] for c in range(C)], F8)

    # ---- L4: H3 = s*Agg(T3)/M3 + b3 ----
    r4 = run(_prog_agg(meta, 4),
             [{"MSG": _stage_msgs(T3, prep["srcmap"][c]),
               "SA": np.ascontiguousarray(prep["SA4"][c]), "BK": B3r,
               "ID": IDENT8}
              for c in range(C)])
    H3 = _assemble(prep,
                   [np.asarray(r4[c]["OUT"])[0:HID] for c in range(C)],
                   [np.asarray(r4[c]["OUT"])[HID:P] for c in range(C)], F16)
    H3[N] = np.float16(-60000.0)     # pad row for the pool staging

    # ---- L5: per-graph max pool ----
    r5 = run(_prog_pool(meta),
             [{"MSGP": _stage_msgs(H3, prep["poolmap"][c])}
              for c in range(C)])
    out = np.concatenate([np.asarray(r5[c]["OUTF"]).T for c in range(C)],
                         axis=0).astype(np.float32)
    out[prep["cnt"].reshape(-1) == 0] = -np.inf
    return out


# revision 22
# speedup vs baseline: 1.4387x; 1.0195x over previous
"""Trainium2 Bass kernel for a 3-layer GCN encoder with global max pool.

Strategy (8 NeuronCores, SPMD, 5 launches, host staging between launches):
  - Nodes partitioned graph-wise (graph g -> core g//64). The host only MOVES
    device-computed bytes between launches (gather rows into padded message
    tables); every FLOP runs on device.
  - Aggregation layers stage per-edge messages in fp8 (e3m4) with fixed
    power-free scale factors folded into the device-side s-tables, halving
    HBM traffic vs fp16.
  - The aggregation sum runs on the Tensor engine: identity-weight matmuls
    accumulate message strips into PSUM (start/stop prefix accumulation).
    Columns (node pairs) are sorted by descending in-degree so the set of
    columns with a message at depth d is a prefix; strips are stored
    region-major (one 512-column PSUM bank region at a time) so PSUM holds
    each region until its sum completes.
  - Post-ops per region: DVE multiplies PSUM by the s-table; Activation
    applies (scaled) bias+relu; DVE writes the next layer's pre-scaled
    fp8 table directly.
  - Launches: L1  T1 = (s*M1) * (X @ W1)                 [fp8 out]
              L2  T2 = (s*M2) * relu(s*Agg(T1)/M1 + b1)  [fp8 out]
              L3  T3 = (s*M3) * (relu((s*Agg(T2)/M2)@W2 + b2)@W3)
              L4  H3 = s*Agg(T3)/M3 + b3                 [fp16 out]
              L5  per-graph max pool over H3 (depth-major staged layout)
"""

import numpy as np
import ml_dtypes

N = 50000
IN_DIM = 128
HID = 64
F2 = 2 * HID
N_GRAPHS = 512
C = 8
P = 128
GPC = N_GRAPHS // C
RW = 512            # psum region width (columns)
F16 = np.float16
F8 = ml_dtypes.float8_e3m4

M1, M2, M3 = 5.0, 11.0, 44.0   # staging scale factors (fold into s-tables)

# L3 aggregation offload: region -> (PE strip count, engine for the rest)
_L3_OFF = {}


# --------------------------------------------------------------------------
# Host-side preprocessing (graph structure only - no feature arithmetic)
# --------------------------------------------------------------------------

def _host_prep(edge_index, batch):
    src = np.asarray(edge_index[0], dtype=np.int64)
    dst = np.asarray(edge_index[1], dtype=np.int64)
    batch = np.asarray(batch, dtype=np.int64)
    core_of = batch // GPC

    indeg = np.bincount(dst, minlength=N)
    k = indeg + 1                     # slots per node incl. self loop
    s = (1.0 / np.sqrt(k.astype(np.float64))).astype(np.float32)

    # in-neighbor lists grouped by dst
    eorder = np.argsort(dst, kind="stable")
    esrc = src[eorder]
    estart = np.zeros(N + 1, np.int64)
    np.cumsum(np.bincount(dst, minlength=N), out=estart[1:])

    # per-core node order: descending k, paired (2i, 2i+1) into columns
    orders = []
    for c in range(C):
        nodes = np.nonzero(core_of == c)[0]
        orders.append(nodes[np.argsort(-k[nodes], kind="stable")])
    NCOL = max((len(o) + 1) // 2 for o in orders)

    tops = np.full((C, NCOL), -1, np.int64)
    bots = np.full((C, NCOL), -1, np.int64)
    for c in range(C):
        o = orders[c]
        tops[c, : len(o[0::2])] = o[0::2]
        bots[c, : len(o[1::2])] = o[1::2]
    topsx = np.where(tops >= 0, tops, N)
    botsx = np.where(bots >= 0, bots, N)

    # column depth = max over cores of max(k_top, k_bot); >=1 (self loop)
    kk = np.concatenate([k, [0]])
    D_col = np.maximum(kk[topsx], kk[botsx]).max(axis=0)
    D_col = np.maximum(D_col, 1)
    assert (np.diff(D_col) <= 0).all()
    DMAX = int(D_col[0])
    n_d = np.array([(D_col > d).sum() for d in range(DMAX)], np.int64)

    # region-major strips: (region_col0, d, w, slot_off)
    strips = []
    off = 0
    regions = []          # (col0, width, D_r, [strip indices])
    for r0 in range(0, NCOL, RW):
        wr = min(RW, NCOL - r0)
        Dr = int(D_col[r0])
        idxs = []
        for d in range(Dr):
            w = int(min(n_d[d] - r0, wr))
            assert w > 0
            idxs.append(len(strips))
            strips.append((r0, d, w, off))
            off += w
        regions.append((r0, wr, Dr, idxs))
    SLOTS = off

    # slot -> source node maps (N = zero row) for tops/bottoms
    indegx = np.concatenate([indeg, [0]])
    estartx = np.concatenate([estart[:-1], [0]])
    srcmap = np.full((C, 2, SLOTS), N, np.int64)
    for (r0, d, w, soff) in strips:
        for c in range(C):
            for half, nodes_h in ((0, topsx[c]), (1, botsx[c])):
                v = nodes_h[r0 : r0 + w]
                if d == 0:
                    srcmap[c, half, soff : soff + w] = v
                else:
                    sel = (d <= indegx[v]) & (v < N)
                    tgt = srcmap[c, half, soff : soff + w]
                    tgt[sel] = esrc[estartx[v[sel]] + d - 1]

    # s tables [C, 128, NCOL] fp16: top s in rows 0:64, bot s in rows 64:128
    def s_table(scale):
        sx = np.concatenate([s * scale, [0.0]]).astype(F16)
        top = sx[topsx][:, None, :].repeat(HID, axis=1)
        bot = sx[botsx][:, None, :].repeat(HID, axis=1)
        return np.concatenate([top, bot], axis=1)      # [C, 128, NCOL]

    SRT1 = s_table(M1)
    SA2 = s_table(1.0 / M1)
    SA3 = s_table(1.0 / M2)
    SA4 = s_table(1.0 / M3)

    # pooling: two graph-groups, depth-major within each: for group G the
    # column at G*(S2*GH) + d*GH + j holds the d-th node pair of graph
    # G*GH + j  (GH = GPC//2 graphs per group)
    GH = GPC // 2
    gl = batch % GPC
    cnt = np.zeros((C, GPC), np.int64)
    np.add.at(cnt, (core_of, gl), 1)
    S2 = int(-(-cnt.max() // 2))
    poolmap = np.full((C, 2, 2 * S2 * GH), N, np.int64)
    for c in range(C):
        for g in range(GPC):
            nodes = np.nonzero((core_of == c) & (gl == g))[0]
            e = nodes[0::2]
            o = nodes[1::2]
            base = (g // GH) * (S2 * GH) + (g % GH)
            poolmap[c, 0, base : base + S2 * GH : GH][: len(e)] = e
            poolmap[c, 1, base : base + S2 * GH : GH][: len(o)] = o

    meta = dict(NCOL=NCOL, SLOTS=SLOTS, S2=S2, strips=strips,
                regions=regions)
    return dict(meta=meta, tops=tops, bots=bots, topsx=topsx, botsx=botsx,
                srcmap=srcmap, poolmap=poolmap, cnt=cnt,
                SRT1=SRT1, SA2=SA2, SA3=SA3, SA4=SA4)


# --------------------------------------------------------------------------
# Bass programs
# --------------------------------------------------------------------------

def _mk_bass():
    import concourse.bacc as bacc
    return bacc.Bacc(None)


def _msg_chunks(strips, SLOTS, first=1400, later=3600):
    """Split the slot axis into DMA chunks at strip boundaries."""
    cuts = []
    target = first
    for (r0, d, w, soff) in strips:
        end = soff + w
        if end >= target:
            cuts.append(end)
            target = end + later
    if not cuts or cuts[-1] != SLOTS:
        cuts.append(SLOTS)
    out = []
    a = 0
    for b in cuts:
        out.append((a, b))
        a = b
    return out


def _prog_agg(meta, layer):
    """L2 (layer==2): OUT = (s*M2)*relu((M1*M2)*(A*s/M1) + (M1*M2)*b1), fp8
       L4 (layer==4): OUT = A*s/M3 + b3, fp16"""
    import concourse.mybir as mybir
    import concourse.tile as tile
    f8 = mybir.dt.float8e3
    f16 = mybir.dt.float16
    f32 = mybir.dt.float32
    Alu = mybir.AluOpType
    Act = mybir.ActivationFunctionType
    NCOL, SLOTS = meta["NCOL"], meta["SLOTS"]
    nc = _mk_bass()

    MSG_d = nc.dram_tensor("MSG", [P, SLOTS], f8, kind="ExternalInput")
    SA_d = nc.dram_tensor("SA", [P, NCOL], f16, kind="ExternalInput")
    BK_d = nc.dram_tensor("BK", [P, 1], f32, kind="ExternalInput")
    ID_d = nc.dram_tensor("ID", [P, P], f8, kind="ExternalInput")
    OUT_d = nc.dram_tensor("OUT", [P, NCOL], f8 if layer == 2 else f16,
                           kind="ExternalOutput")

    with tile.TileContext(nc, num_cores=C) as tc:
        with (
            tc.tile_pool(name="const", bufs=1) as const,
            tc.tile_pool(name="ps", bufs=3, space="PSUM") as psp,
        ):
            ID_s = const.tile([P, P], f8)
            nc.sync.dma_start(ID_s[:], ID_d[:])
            MSG_s = const.tile([P, SLOTS], f8)
            chunks = _msg_chunks(meta["strips"], SLOTS)
            SA_s = const.tile([P, NCOL], f16)
            BK_s = const.tile([P, 1], f32)
            for ci, (a, b) in enumerate(chunks):
                nc.sync.dma_start(MSG_s[:, a:b], MSG_d[:, a:b])
                if ci == 2:
                    nc.sync.dma_start(SA_s[:], SA_d[:])
                    nc.sync.dma_start(BK_s[:], BK_d[:])
            U_s = const.tile([P, NCOL], f16)
            H_s = const.tile([P, NCOL], f16)
            OUT_s = const.tile([P, NCOL], f8 if layer == 2 else f16)

            # PE warm-up during the first MSG DMA (pstate ramp)
            wm = psp.tile([P, RW], f32, tag="ps")
            for i in range(8):
                nc.tensor.matmul(wm[:, :P], lhsT=ID_s[:], rhs=ID_s[:],
                                 start=(i == 0), stop=(i == 7))

            nreg = len(meta["regions"])
            stored = [0]

            def fin(ri):
                # final OUT mult + store, emitted one region late so the
                # engines pipeline across the last two regions
                (r0, wr, Dr, idxs) = meta["regions"][ri]
                cols = slice(r0, r0 + wr)
                if layer == 2:
                    eng = nc.gpsimd if ri in (1, 3) else nc.vector
                    eng.tensor_tensor(out=OUT_s[:, cols], in0=H_s[:, cols],
                                      in1=SA_s[:, cols], op=Alu.mult)
                if ri % 2 == 1 or ri >= nreg - 3 or r0 + wr == NCOL:
                    nc.sync.dma_start(OUT_d[:, stored[0] : r0 + wr],
                                      OUT_s[:, stored[0] : r0 + wr])
                    stored[0] = r0 + wr

            for ri, (r0, wr, Dr, idxs) in enumerate(meta["regions"]):
                ps = psp.tile([P, RW], f32, tag="ps")
                for j, si in enumerate(idxs):
                    (_, d, w, soff) = meta["strips"][si]
                    nc.tensor.matmul(ps[:, :w], lhsT=ID_s[:],
                                     rhs=MSG_s[:, soff : soff + w],
                                     start=(j == 0), stop=(j == Dr - 1))
                cols = slice(r0, r0 + wr)
                nc.vector.tensor_tensor(out=U_s[:, cols], in0=ps[:, :wr],
                                        in1=SA_s[:, cols], op=Alu.mult)
                if layer == 2:
                    nc.scalar.activation(out=H_s[:, cols], in_=U_s[:, cols],
                                         func=Act.Relu, bias=BK_s[:],
                                         scale=float(M1 * M2))
                else:
                    nc.scalar.activation(out=OUT_s[:, cols], in_=U_s[:, cols],
                                         func=Act.Identity, bias=BK_s[:],
                                         scale=1.0)
                if ri >= 1:
                    fin(ri - 1)
            fin(nreg - 1)
    nc.compile()
    return nc


def _prog_l1(meta):
    """T1 = (s*M1) * (X @ W1), two [64, NCOL] fp8 half strips."""
    import concourse.mybir as mybir
    import concourse.tile as tile
    f8 = mybir.dt.float8e3
    f16 = mybir.dt.float16
    f32 = mybir.dt.float32
    Alu = mybir.AluOpType
    NCOL = meta["NCOL"]
    nc = _mk_bass()

    XT_d = nc.dram_tensor("XT", [IN_DIM, 2 * NCOL], f16, kind="ExternalInput")
    W1_d = nc.dram_tensor("W1", [IN_DIM, HID], f16, kind="ExternalInput")
    SRT1_d = nc.dram_tensor("SRT1", [P, NCOL], f16, kind="ExternalInput")
    T1P_d = nc.dram_tensor("T1P", [P, NCOL], f8, kind="ExternalOutput")

    with tile.TileContext(nc, num_cores=C) as tc:
        with (
            tc.tile_pool(name="const", bufs=1) as const,
            tc.tile_pool(name="ps", bufs=4, space="PSUM") as psp,
        ):
            W1_s = const.tile([IN_DIM, HID], f16)
            nc.sync.dma_start(W1_s[:], W1_d[:])
            XT_s = const.tile([IN_DIM, 2 * NCOL], f16)
            SRT1_s = const.tile([P, NCOL], f16)
            # top-strip chunk0 and bot-strip chunk0 first so the first
            # packed column-pair flows start early; SRT1 deferred
            cuts = [RW, 2 * RW] + list(range(4 * RW, NCOL, 4 * RW)) + [NCOL]
            chunks = []
            a = 0
            for b in cuts:
                if b > a:
                    chunks.append((a, b))
                    a = b
            for ci, (a, b) in enumerate(chunks):
                nc.sync.dma_start(XT_s[:, a:b], XT_d[:, a:b])
                nc.sync.dma_start(XT_s[:, NCOL + a : NCOL + b],
                                  XT_d[:, NCOL + a : NCOL + b])
                if ci == 0:
                    nc.sync.dma_start(SRT1_s[:], SRT1_d[:])
            T1_s = const.tile([P, NCOL], f8)

            # brief PE warm-up while chunk0 loads
            wm = psp.tile([P, RW], f32, tag="ps")
            for i in range(4):
                nc.tensor.matmul(wm[0:HID, :HID], lhsT=W1_s[:], rhs=W1_s[:],
                                 start=(i == 0), stop=(i == 3))

            stored = 0
            nflows = -(-NCOL // RW)
            for i in range(nflows):
                a = i * RW
                w = min(RW, NCOL - a)
                ps = psp.tile([P, RW], f32, tag="ps")
                for half in range(2):
                    nc.tensor.matmul(
                        ps[half * HID : half * HID + HID, :w], lhsT=W1_s[:],
                        rhs=XT_s[:, half * NCOL + a : half * NCOL + a + w],
                        start=True, stop=True)
                nc.vector.tensor_tensor(
                    out=T1_s[:, a : a + w], in0=ps[:, :w],
                    in1=SRT1_s[:, a : a + w], op=Alu.mult)
                if i % 2 == 1 or a + w == NCOL:
                    nc.scalar.dma_start(T1P_d[:, stored : a + w],
                                        T1_s[:, stored : a + w])
                    stored = a + w
    nc.compile()
    return nc


def _prog_l3(meta):
    """T3 = (s*M3)*(relu((s*Agg(T2)/M2)@W2 + b2)@W3), fp8 half strips.
    W3 input is pre-scaled by M2*M3 on the host."""
    import concourse.mybir as mybir
    import concourse.tile as tile
    f8 = mybir.dt.float8e3
    f16 = mybir.dt.float16
    f32 = mybir.dt.float32
    Alu = mybir.AluOpType
    Act = mybir.ActivationFunctionType
    NCOL, SLOTS = meta["NCOL"], meta["SLOTS"]
    nc = _mk_bass()

    MSG_d = nc.dram_tensor("MSG", [P, SLOTS], f8, kind="ExternalInput")
    SA_d = nc.dram_tensor("SA", [P, NCOL], f16, kind="ExternalInput")
    W2_d = nc.dram_tensor("W2", [P, F2], f16, kind="ExternalInput")
    W3_d = nc.dram_tensor("W3", [F2, HID], f16, kind="ExternalInput")
    B2_d = nc.dram_tensor("B2", [F2, 1], f32, kind="ExternalInput")
    ID_d = nc.dram_tensor("ID", [P, P], f8, kind="ExternalInput")
    T3P_d = nc.dram_tensor("T3P", [P, NCOL], f8, kind="ExternalOutput")

    # deep strips of these regions are tree-summed on DVE/Pool into an fp16
    # partial; PE folds the partial into PSUM with one extra pass
    OFF = dict(_L3_OFF)

    with tile.TileContext(nc, num_cores=C) as tc:
        with (
            tc.tile_pool(name="const", bufs=1) as const,
            tc.tile_pool(name="psA", bufs=2, space="PSUM") as psAp,
            tc.tile_pool(name="ps2", bufs=4, space="PSUM") as ps2p,
            tc.tile_pool(name="ps3", bufs=2, space="PSUM") as ps3p,
        ):
            ID_s = const.tile([P, P], f8)
            nc.sync.dma_start(ID_s[:], ID_d[:])
            MSG_s = const.tile([P, SLOTS], f8)
            chunks = _msg_chunks(meta["strips"], SLOTS)
            SA_s = const.tile([P, NCOL], f16)
            W2_s = const.tile([P, F2], f16)
            W3_s = const.tile([F2, HID], f16)
            B2_s = const.tile([F2, 1], f32)
            for ci, (a, b) in enumerate(chunks):
                nc.sync.dma_start(MSG_s[:, a:b], MSG_d[:, a:b])
                if ci == 0:
                    nc.sync.dma_start(W2_s[:], W2_d[:])
                    nc.sync.dma_start(W3_s[:], W3_d[:])
                    nc.sync.dma_start(B2_s[:], B2_d[:])
                elif ci == 2:
                    nc.sync.dma_start(SA_s[:], SA_d[:])
            U_s = const.tile([P, NCOL], f16)
            H2E = const.tile([F2, NCOL], f16)
            H2O = const.tile([F2, NCOL], f16)
            T3_s = const.tile([P, NCOL], f8)
            ACC = const.tile([P, NCOL], f16)

            # PE warm-up during the first MSG DMA
            wm = psAp.tile([P, RW], f32, tag="psA")
            for i in range(8):
                nc.tensor.matmul(wm[:, :P], lhsT=ID_s[:], rhs=ID_s[:],
                                 start=(i == 0), stop=(i == 7))

            stored = [0]
            nreg = len(meta["regions"])

            def stage_a(ri):
                (r0, wr, Dr, idxs) = meta["regions"][ri]
                ps = ps_of[ri]
                cols = slice(r0, r0 + wr)
                nc.vector.tensor_tensor(out=U_s[:, cols], in0=ps[:, :wr],
                                        in1=SA_s[:, cols], op=Alu.mult)
                ps2s = []
                for hb, tp in ((0, (0, 0)), (HID, (HID, 0))):
                    ps2 = ps2p.tile([F2, RW], f32, tag="ps2")
                    nc.tensor.matmul(ps2[:, :wr],
                                     lhsT=W2_s[hb : hb + HID, :],
                                     rhs=U_s[hb : hb + HID, cols],
                                     start=True, stop=True, tile_position=tp)
                    ps2s.append(ps2)
                ps2_of[ri] = ps2s
                for ps2, H2 in zip(ps2s, (H2E, H2O)):
                    nc.scalar.activation(out=H2[:, cols], in_=ps2[:, :wr],
                                         func=Act.Relu, bias=B2_s[:],
                                         scale=1.0)

            def stage_c(ri):
                (r0, wr, Dr, idxs) = meta["regions"][ri]
                cols = slice(r0, r0 + wr)
                ps3 = ps3p.tile([P, RW], f32, tag="ps3")
                for hb, H2 in ((0, H2E), (HID, H2O)):
                    nc.tensor.matmul(ps3[hb : hb + HID, :wr], lhsT=W3_s[:],
                                     rhs=H2[:, cols], start=True, stop=True)
                nc.vector.tensor_tensor(out=T3_s[:, cols], in0=ps3[:, :wr],
                                        in1=SA_s[:, cols], op=Alu.mult)
                if ri % 2 == 1 or ri >= nreg - 3 or r0 + wr == NCOL:
                    nc.sync.dma_start(T3P_d[:, stored[0] : r0 + wr],
                                      T3_s[:, stored[0] : r0 + wr])
                    stored[0] = r0 + wr

            ps_of = {}
            ps2_of = {}
            for ri, (r0, wr, Dr, idxs) in enumerate(meta["regions"]):
                split, eng_name = OFF.get(ri, (Dr, None))
                split = min(split, Dr)
                if split < Dr:
                    eng = nc.vector if eng_name == "dve" else nc.gpsimd
                    first = True
                    for si in idxs[split:]:
                        (_, d, w, soff) = meta["strips"][si]
                        strip = MSG_s[:, soff : soff + w]
                        if first:
                            eng.tensor_copy(ACC[:, r0 : r0 + w], strip)
                            first = False
                        else:
                            eng.tensor_tensor(out=ACC[:, r0 : r0 + w],
                                              in0=ACC[:, r0 : r0 + w],
                                              in1=strip, op=Alu.add)
                ps = psAp.tile([P, RW], f32, tag="psA")
                ps_of[ri] = ps
                for j, si in enumerate(idxs[:split]):
                    (_, d, w, soff) = meta["strips"][si]
                    nc.tensor.matmul(ps[:, :w], lhsT=ID_s[:],
                                     rhs=MSG_s[:, soff : soff + w],
                                     start=(j == 0),
                                     stop=(j == Dr - 1 and split == Dr))
                if split < Dr:
                    (_, d, w, soff) = meta["strips"][idxs[split]]
                    nc.tensor.matmul(ps[:, :w], lhsT=ID_s[:],
                                     rhs=ACC[:, r0 : r0 + w],
                                     start=False, stop=True)
                if ri >= 1:
                    stage_a(ri - 1)
                if ri >= 2:
                    stage_c(ri - 2)
            stage_a(nreg - 1)
            stage_c(nreg - 2)
            stage_c(nreg - 1)
    nc.compile()
    return nc


def _prog_pool(meta):
    """Per-graph max over two depth-major groups [128, 2*(S2*GH)] fp16;
    OUTF [HID, GPC] f32 is features x graphs (host transposes)."""
    import concourse.mybir as mybir
    import concourse.tile as tile
    f16 = mybir.dt.float16
    f32 = mybir.dt.float32
    Alu = mybir.AluOpType
    S2 = meta["S2"]
    GH = GPC // 2
    nc = _mk_bass()

    MSGP_d = nc.dram_tensor("MSGP", [P, 2 * S2 * GH], f16,
                            kind="ExternalInput")
    OUTF_d = nc.dram_tensor("OUTF", [HID, GPC], f32, kind="ExternalOutput")

    with tile.TileContext(nc, num_cores=C) as tc:
        with (
            tc.tile_pool(name="const", bufs=1) as const,
            tc.tile_pool(name="ps", bufs=2, space="PSUM") as psp,
        ):
            mg = const.tile([P, 2 * S2 * GH], f16)
            OUTF_s = const.tile([HID, GPC], f32)
            for grp in range(2):
                g0 = grp * S2 * GH
                nc.sync.dma_start(mg[:, g0 : g0 + S2 * GH],
                                  MSGP_d[:, g0 : g0 + S2 * GH])
            for grp in range(2):
                g0 = grp * S2 * GH
                v = mg[:, g0 : g0 + S2 * GH].rearrange(
                    "p (d g) -> p d g", g=GH)
                cur = S2
                while cur > 1:
                    h = cur // 2
                    if cur % 2:
                        nc.vector.tensor_tensor(
                            out=v[:, 0, :], in0=v[:, 0, :],
                            in1=v[:, cur - 1, :], op=Alu.max)
                    nc.vector.tensor_tensor(
                        out=v[:, 0:h, :], in0=v[:, 0:h, :],
                        in1=v[:, h : 2 * h, :], op=Alu.max)
                    cur = h
                bot = psp.tile([HID, GH], f32, tag="bot")
                nc.vector.tensor_copy(bot[:], mg[HID:P, g0 : g0 + GH])
                nc.vector.tensor_tensor(
                    out=OUTF_s[:, grp * GH : (grp + 1) * GH],
                    in0=mg[0:HID, g0 : g0 + GH],
                    in1=bot[:], op=Alu.max)
            nc.sync.dma_start(OUTF_d[:], OUTF_s[:])
    nc.compile()
    return nc


# --------------------------------------------------------------------------
# Entry point
# --------------------------------------------------------------------------

_RUN_KWARGS = {}
_EXEC_NS = []
_PROFILE = False


def _stage_msgs(T_full, srcmap_c):
    """[N+1, HID] table + [2, SLOTS] slot->row map -> [128, SLOTS]."""
    top = T_full[srcmap_c[0]].T      # [64, SLOTS]
    bot = T_full[srcmap_c[1]].T
    return np.ascontiguousarray(np.concatenate([top, bot], axis=0))


def _assemble(prep, parts_E, parts_O, dtype):
    """Per-core [64, NCOL] half strips -> full [N+1, HID] table."""
    T_full = np.zeros((N + 1, HID), dtype)
    for c in range(C):
        tops, bots = prep["tops"][c], prep["bots"][c]
        mE, mO = tops >= 0, bots >= 0
        T_full[tops[mE]] = parts_E[c][:, mE].T
        T_full[bots[mO]] = parts_O[c][:, mO].T
    return T_full


def kernel(data, edge_index, batch, W1, b1, W2, b2, W3, b3):
    from concourse.bass_utils import run_bass_kernel_spmd

    data = np.asarray(data, dtype=np.float32)
    edge_index = np.asarray(edge_index, dtype=np.int32)
    batch_np = np.asarray(batch, dtype=np.int32)

    prep = _host_prep(edge_index, batch_np)
    meta = prep["meta"]
    NCOL = meta["NCOL"]

    IDENT8 = np.eye(P, dtype=F8)
    W1f = np.asarray(W1, np.float32).astype(F16)            # [128, 64]
    W2f = np.asarray(W2, np.float32).astype(F16)            # [64, 128]
    W3f = (np.asarray(W3, np.float32) * (M2 * M3)).astype(F16)
    B1K = (np.tile(np.asarray(b1, np.float32), 2) * (M1 * M2))[:, None].copy()
    B2r = np.asarray(b2, np.float32)[:, None].copy()
    B3r = np.tile(np.asarray(b3, np.float32), 2)[:, None].copy()

    Xx = np.concatenate([data, np.zeros((1, IN_DIM), np.float32)], axis=0)
    XT = np.empty((C, IN_DIM, 2 * NCOL), F16)
    for c in range(C):
        XT[c, :, :NCOL] = Xx[prep["topsx"][c]].T
        XT[c, :, NCOL:] = Xx[prep["botsx"][c]].T

    cores = list(range(C))
    del _EXEC_NS[:]

    def run(nc, in_maps):
        if _PROFILE:
            from concourse.timeline_sim import TimelineSim
            _EXEC_NS.append(TimelineSim(nc, require_finite=False).simulate())
        res = run_bass_kernel_spmd(nc, in_maps, cores, **_RUN_KWARGS)
        if res.exec_time_ns is not None:
            _EXEC_NS.append(res.exec_time_ns)
        return res.results

    # ---- L1: T1 = (s*M1) * (X @ W1) ----
    r1 = run(_prog_l1(meta),
             [{"XT": np.ascontiguousarray(XT[c]), "W1": W1f,
               "SRT1": np.ascontiguousarray(prep["SRT1"][c])}
              for c in range(C)])
    T1 = _assemble(prep,
                   [np.asarray(r1[c]["T1P"])[0:HID] for c in range(C)],
                   [np.asarray(r1[c]["T1P"])[HID:P] for c in range(C)], F8)

    # ---- L2: T2 = (s*M2)*relu(s*Agg(T1)/M1 + b1) ----
    r2 = run(_prog_agg(meta, 2),
             [{"MSG": _stage_msgs(T1, prep["srcmap"][c]),
               "SA": np.ascontiguousarray(prep["SA2"][c]), "BK": B1K,
               "ID": IDENT8}
              for c in range(C)])
    T2 = _assemble(prep,
                   [np.asarray(r2[c]["OUT"])[0:HID] for c in range(C)],
                   [np.asarray(r2[c]["OUT"])[HID:P] for c in range(C)], F8)

    # ---- L3: T3 = (s*M3)*(relu((s*Agg(T2)/M2)@W2 + b2)@W3) ----
    r3 = run(_prog_l3(meta),
             [{"MSG": _stage_msgs(T2, prep["srcmap"][c]),
               "SA": np.ascontiguousarray(prep["SA3"][c]),
               "W2": np.concatenate([W2f, W2f], axis=0), "W3": W3f,
               "B2": B2r, "ID": IDENT8}
              for c in range(C)])
    T3 = _assemble(prep,
                   [np.asarray(r3[c]["T3P"])[0:HID] for c in range(C)],
                   [np.asarray(r3[c]["T3P"])[HID:P] for c in range(C)], F8)

    # ---- L4: H3 = s*Agg(T3)/M3 + b3 ----
    r4 = run(_prog_agg(meta, 4),
             [{"MSG": _stage_msgs(T3, prep["srcmap"][c]),
               "SA": np.ascontiguousarray(prep["SA4"][c]), "BK": B3r,
               "ID": IDENT8}
              for c in range(C)])
    H3 = _assemble(prep,
                   [np.asarray(r4[c]["OUT"])[0:HID] for c in range(C)],
                   [np.asarray(r4[c]["OUT"])[HID:P] for c in range(C)], F16)
    H3[N] = np.float16(-60000.0)     # pad row for the pool staging

    # ---- L5: per-graph max pool ----
    r5 = run(_prog_pool(meta),
             [{"MSGP": _stage_msgs(H3, prep["poolmap"][c])}
              for c in range(C)])
    out = np.concatenate([np.asarray(r5[c]["OUTF"]).T for c in range(C)],
                         axis=0).astype(np.float32)
    out[prep["cnt"].reshape(-1) == 0] = -np.inf
    return out


# revision 31
# speedup vs baseline: 1.4536x; 1.0104x over previous
"""Trainium2 Bass kernel for a 3-layer GCN encoder with global max pool.

Strategy (8 NeuronCores, SPMD, 5 launches, host staging between launches):
  - Nodes partitioned graph-wise (graph g -> core g//64). The host only MOVES
    device-computed bytes between launches (gather rows into padded message
    tables); every FLOP runs on device.
  - Aggregation layers stage per-edge messages in fp8 (e3m4) with fixed
    power-free scale factors folded into the device-side s-tables, halving
    HBM traffic vs fp16.
  - The aggregation sum runs on the Tensor engine: identity-weight matmuls
    accumulate message strips into PSUM (start/stop prefix accumulation).
    Columns (node pairs) are sorted by descending in-degree so the set of
    columns with a message at depth d is a prefix; strips are stored
    region-major (one 512-column PSUM bank region at a time) so PSUM holds
    each region until its sum completes.
  - Post-ops per region: DVE multiplies PSUM by the s-table; Activation
    applies (scaled) bias+relu; DVE writes the next layer's pre-scaled
    fp8 table directly.
  - Launches: L1  T1 = (s*M1) * (X @ W1)                 [fp8 out]
              L2  T2 = (s*M2) * relu(s*Agg(T1)/M1 + b1)  [fp8 out]
              L3  T3 = (s*M3) * (relu((s*Agg(T2)/M2)@W2 + b2)@W3)
              L4  H3 = s*Agg(T3)/M3 + b3                 [fp16 out]
              L5  per-graph max pool over H3 (depth-major staged layout)
"""

import numpy as np
import ml_dtypes

N = 50000
IN_DIM = 128
HID = 64
F2 = 2 * HID
N_GRAPHS = 512
C = 8
P = 128
GPC = N_GRAPHS // C
RW = 512            # psum region width (columns)
F16 = np.float16
F8 = ml_dtypes.float8_e3m4

M1, M2, M3 = 5.0, 11.0, 44.0   # staging scale factors (fold into s-tables)

# L3 aggregation offload: region -> (PE strip count, engine for the rest)
_L3_OFF = {1: (10, "dve"), 2: (12, "pool"), 3: (10, "dve")}


# --------------------------------------------------------------------------
# Host-side preprocessing (graph structure only - no feature arithmetic)
# --------------------------------------------------------------------------

def _host_prep(edge_index, batch):
    src = np.asarray(edge_index[0], dtype=np.int64)
    dst = np.asarray(edge_index[1], dtype=np.int64)
    batch = np.asarray(batch, dtype=np.int64)
    core_of = batch // GPC

    indeg = np.bincount(dst, minlength=N)
    k = indeg + 1                     # slots per node incl. self loop
    s = (1.0 / np.sqrt(k.astype(np.float64))).astype(np.float32)

    # in-neighbor lists grouped by dst
    eorder = np.argsort(dst, kind="stable")
    esrc = src[eorder]
    estart = np.zeros(N + 1, np.int64)
    np.cumsum(np.bincount(dst, minlength=N), out=estart[1:])

    # per-core node order: descending k, paired (2i, 2i+1) into columns
    orders = []
    for c in range(C):
        nodes = np.nonzero(core_of == c)[0]
        orders.append(nodes[np.argsort(-k[nodes], kind="stable")])
    NCOL = max((len(o) + 1) // 2 for o in orders)

    tops = np.full((C, NCOL), -1, np.int64)
    bots = np.full((C, NCOL), -1, np.int64)
    for c in range(C):
        o = orders[c]
        tops[c, : len(o[0::2])] = o[0::2]
        bots[c, : len(o[1::2])] = o[1::2]
    topsx = np.where(tops >= 0, tops, N)
    botsx = np.where(bots >= 0, bots, N)

    # column depth = max over cores of max(k_top, k_bot); >=1 (self loop)
    kk = np.concatenate([k, [0]])
    D_col = np.maximum(kk[topsx], kk[botsx]).max(axis=0)
    D_col = np.maximum(D_col, 1)
    assert (np.diff(D_col) <= 0).all()
    DMAX = int(D_col[0])
    n_d = np.array([(D_col > d).sum() for d in range(DMAX)], np.int64)

    # region widths: full PSUM banks, with the remainder split into a
    # shrinking taper so the final post-op chains are short
    widths = []
    rem = NCOL
    while rem > 704:
        widths.append(RW)
        rem -= RW
    if rem > 384:
        w1 = (rem * 33 // 64) & ~31
        w2 = ((rem - w1) * 3 // 5) & ~31
        widths += [w1, w2, rem - w1 - w2]
    elif rem > 160:
        w1 = (rem * 3 // 5) & ~31
        widths += [w1, rem - w1]
    else:
        widths.append(rem)

    # region-major strips: (region_col0, d, w, slot_off)
    strips = []
    off = 0
    regions = []          # (col0, width, D_r, [strip indices])
    r0 = 0
    for wr in widths:
        Dr = int(D_col[r0])
        idxs = []
        for d in range(Dr):
            w = int(min(n_d[d] - r0, wr))
            assert w > 0
            idxs.append(len(strips))
            strips.append((r0, d, w, off))
            off += w
        regions.append((r0, wr, Dr, idxs))
        r0 += wr
    SLOTS = off

    # slot -> source node maps (N = zero row) for tops/bottoms
    indegx = np.concatenate([indeg, [0]])
    estartx = np.concatenate([estart[:-1], [0]])
    srcmap = np.full((C, 2, SLOTS), N, np.int64)
    for (r0, d, w, soff) in strips:
        for c in range(C):
            for half, nodes_h in ((0, topsx[c]), (1, botsx[c])):
                v = nodes_h[r0 : r0 + w]
                if d == 0:
                    srcmap[c, half, soff : soff + w] = v
                else:
                    sel = (d <= indegx[v]) & (v < N)
                    tgt = srcmap[c, half, soff : soff + w]
                    tgt[sel] = esrc[estartx[v[sel]] + d - 1]

    # s tables [C, 128, NCOL] fp16: top s in rows 0:64, bot s in rows 64:128
    def s_table(scale):
        sx = np.concatenate([s * scale, [0.0]]).astype(F16)
        top = sx[topsx][:, None, :].repeat(HID, axis=1)
        bot = sx[botsx][:, None, :].repeat(HID, axis=1)
        return np.concatenate([top, bot], axis=1)      # [C, 128, NCOL]

    SRT1 = s_table(M1)
    SA2 = s_table(1.0 / M1)
    SA3 = s_table(1.0 / M2)
    SA4 = s_table(1.0 / M3)

    # pooling: graphs ranked by size per core, split into NPG groups of GH;
    # group G is depth-major: column offG + d*GH + j = d-th node pair of the
    # (G*GH+j)-th largest graph.  Per-group depth S2G trims the rectangle.
    NPG = 4
    GH = GPC // NPG
    gl = batch % GPC
    cnt = np.zeros((C, GPC), np.int64)
    np.add.at(cnt, (core_of, gl), 1)
    grank = np.argsort(-cnt, axis=1, kind="stable")     # [C, GPC] rank->graph
    pairs = -(-cnt // 2)
    S2G = []
    for G in range(NPG):
        S2G.append(int(max(pairs[c, grank[c, G * GH]] for c in range(C))))
    offG = np.zeros(NPG + 1, np.int64)
    np.cumsum(np.array(S2G) * GH, out=offG[1:])
    POOLW = int(offG[-1])
    poolmap = np.full((C, 2, POOLW), N, np.int64)
    for c in range(C):
        for j in range(GPC):
            g = grank[c, j]
            nodes = np.nonzero((core_of == c) & (gl == g))[0]
            e = nodes[0::2]
            o = nodes[1::2]
            G = j // GH
            base = int(offG[G]) + (j % GH)
            poolmap[c, 0, base : base + S2G[G] * GH : GH][: len(e)] = e
            poolmap[c, 1, base : base + S2G[G] * GH : GH][: len(o)] = o

    meta = dict(NCOL=NCOL, SLOTS=SLOTS, strips=strips, regions=regions,
                NPG=NPG, GH=GH, S2G=S2G, offG=[int(x) for x in offG],
                POOLW=POOLW)
    return dict(meta=meta, tops=tops, bots=bots, topsx=topsx, botsx=botsx,
                srcmap=srcmap, poolmap=poolmap, cnt=cnt, grank=grank,
                SRT1=SRT1, SA2=SA2, SA3=SA3, SA4=SA4)


# --------------------------------------------------------------------------
# Bass programs
# --------------------------------------------------------------------------

def _mk_bass():
    import concourse.bacc as bacc
    return bacc.Bacc(None)


def _msg_chunks(strips, SLOTS, first=1400, later=3600):
    """Split the slot axis into DMA chunks at strip boundaries."""
    cuts = []
    target = first
    for (r0, d, w, soff) in strips:
        end = soff + w
        if end >= target:
            cuts.append(end)
            target = end + later
    if not cuts or cuts[-1] != SLOTS:
        cuts.append(SLOTS)
    out = []
    a = 0
    for b in cuts:
        out.append((a, b))
        a = b
    return out


def _prog_agg(meta, layer):
    """L2 (layer==2): OUT = (s*M2)*relu((M1*M2)*(A*s/M1) + (M1*M2)*b1), fp8
       L4 (layer==4): OUT = A*s/M3 + b3, fp16"""
    import concourse.mybir as mybir
    import concourse.tile as tile
    f8 = mybir.dt.float8e3
    f16 = mybir.dt.float16
    f32 = mybir.dt.float32
    Alu = mybir.AluOpType
    Act = mybir.ActivationFunctionType
    NCOL, SLOTS = meta["NCOL"], meta["SLOTS"]
    nc = _mk_bass()

    MSG_d = nc.dram_tensor("MSG", [P, SLOTS], f8, kind="ExternalInput")
    SA_d = nc.dram_tensor("SA", [P, NCOL], f16, kind="ExternalInput")
    BK_d = nc.dram_tensor("BK", [P, 1], f32, kind="ExternalInput")
    ID_d = nc.dram_tensor("ID", [P, P], f8, kind="ExternalInput")
    OUT_d = nc.dram_tensor("OUT", [P, NCOL], f8 if layer == 2 else f16,
                           kind="ExternalOutput")

    with tile.TileContext(nc, num_cores=C) as tc:
        with (
            tc.tile_pool(name="const", bufs=1) as const,
            tc.tile_pool(name="ps", bufs=3, space="PSUM") as psp,
        ):
            ID_s = const.tile([P, P], f8)
            nc.sync.dma_start(ID_s[:], ID_d[:])
            MSG_s = const.tile([P, SLOTS], f8)
            chunks = _msg_chunks(meta["strips"], SLOTS)
            SA_s = const.tile([P, NCOL], f16)
            BK_s = const.tile([P, 1], f32)
            for ci, (a, b) in enumerate(chunks):
                nc.sync.dma_start(MSG_s[:, a:b], MSG_d[:, a:b])
                if ci == 2:
                    nc.sync.dma_start(SA_s[:], SA_d[:])
                    nc.sync.dma_start(BK_s[:], BK_d[:])
            U_s = const.tile([P, NCOL], f16)
            H_s = const.tile([P, NCOL], f16)
            OUT_s = const.tile([P, NCOL], f8 if layer == 2 else f16)

            # PE warm-up during the first MSG DMA (pstate ramp)
            wm = psp.tile([P, RW], f32, tag="ps")
            for i in range(8):
                nc.tensor.matmul(wm[:, :P], lhsT=ID_s[:], rhs=ID_s[:],
                                 start=(i == 0), stop=(i == 7))

            nreg = len(meta["regions"])
            stored = [0]

            def fin(ri):
                # final OUT mult + store, emitted one region late so the
                # engines pipeline across the last two regions
                (r0, wr, Dr, idxs) = meta["regions"][ri]
                cols = slice(r0, r0 + wr)
                if layer == 2:
                    eng = nc.gpsimd if ri in (1, 3) else nc.vector
                    eng.tensor_tensor(out=OUT_s[:, cols], in0=H_s[:, cols],
                                      in1=SA_s[:, cols], op=Alu.mult)
                if ri % 2 == 1 or ri >= nreg - 3 or r0 + wr == NCOL:
                    nc.sync.dma_start(OUT_d[:, stored[0] : r0 + wr],
                                      OUT_s[:, stored[0] : r0 + wr])
                    stored[0] = r0 + wr

            for ri, (r0, wr, Dr, idxs) in enumerate(meta["regions"]):
                ps = psp.tile([P, RW], f32, tag="ps")
                for j, si in enumerate(idxs):
                    (_, d, w, soff) = meta["strips"][si]
                    nc.tensor.matmul(ps[:, :w], lhsT=ID_s[:],
                                     rhs=MSG_s[:, soff : soff + w],
                                     start=(j == 0), stop=(j == Dr - 1))
                cols = slice(r0, r0 + wr)
                nc.vector.tensor_tensor(out=U_s[:, cols], in0=ps[:, :wr],
                                        in1=SA_s[:, cols], op=Alu.mult)
                if layer == 2:
                    nc.scalar.activation(out=H_s[:, cols], in_=U_s[:, cols],
                                         func=Act.Relu, bias=BK_s[:],
                                         scale=float(M1 * M2))
                else:
                    nc.scalar.activation(out=OUT_s[:, cols], in_=U_s[:, cols],
                                         func=Act.Identity, bias=BK_s[:],
                                         scale=1.0)
                if ri >= 1:
                    fin(ri - 1)
            fin(nreg - 1)
    nc.compile()
    return nc


def _prog_l1(meta):
    """T1 = (s*M1) * (X @ W1), two [64, NCOL] fp8 half strips."""
    import concourse.mybir as mybir
    import concourse.tile as tile
    f8 = mybir.dt.float8e3
    f16 = mybir.dt.float16
    f32 = mybir.dt.float32
    Alu = mybir.AluOpType
    Act = mybir.ActivationFunctionType
    NCOL = meta["NCOL"]
    nc = _mk_bass()

    XT_d = nc.dram_tensor("XT", [IN_DIM, 2 * NCOL], f16, kind="ExternalInput")
    W1_d = nc.dram_tensor("W1", [IN_DIM, HID], f16, kind="ExternalInput")
    SRT1_d = nc.dram_tensor("SRT1", [P, NCOL], f16, kind="ExternalInput")
    T1P_d = nc.dram_tensor("T1P", [P, NCOL], f8, kind="ExternalOutput")

    with tile.TileContext(nc, num_cores=C) as tc:
        with (
            tc.tile_pool(name="const", bufs=1) as const,
            tc.tile_pool(name="ps", bufs=4, space="PSUM") as psp,
        ):
            W1_s = const.tile([IN_DIM, HID], f16)
            nc.sync.dma_start(W1_s[:], W1_d[:])
            XT_s = const.tile([IN_DIM, 2 * NCOL], f16)
            SRT1_s = const.tile([P, NCOL], f16)
            # interleave XT flow chunks (top+bot pair) with SRT1 chunks so
            # the DVE mults are never gated on a monolithic SRT1 load
            cuts = [RW] + list(range(2 * RW, NCOL, 2 * RW)) + [NCOL]
            chunks = []
            a = 0
            for b in cuts:
                if b > a:
                    chunks.append((a, b))
                    a = b
            for ci, (a, b) in enumerate(chunks):
                nc.sync.dma_start(XT_s[:, a:b], XT_d[:, a:b])
                nc.sync.dma_start(XT_s[:, NCOL + a : NCOL + b],
                                  XT_d[:, NCOL + a : NCOL + b])
                nc.sync.dma_start(SRT1_s[:, a:b], SRT1_d[:, a:b])
            T1_s = const.tile([P, NCOL], f8)
            V_s = const.tile([P, NCOL], f16)

            # brief PE warm-up while chunk0 loads
            wm = psp.tile([P, RW], f32, tag="ps")
            for i in range(4):
                nc.tensor.matmul(wm[0:HID, :HID], lhsT=W1_s[:], rhs=W1_s[:],
                                 start=(i == 0), stop=(i == 3))

            stored = 0
            nflows = -(-NCOL // RW)
            for i in range(nflows):
                a = i * RW
                w = min(RW, NCOL - a)
                ps = psp.tile([P, RW], f32, tag="ps")
                for half in range(2):
                    nc.tensor.matmul(
                        ps[half * HID : half * HID + HID, :w], lhsT=W1_s[:],
                        rhs=XT_s[:, half * NCOL + a : half * NCOL + a + w],
                        start=True, stop=True)
                if i in (2, 4):
                    # relieve DVE: Act copies PSUM out, Pool applies the scale
                    nc.scalar.activation(out=V_s[:, a : a + w],
                                         in_=ps[:, :w], func=Act.Copy,
                                         bias=0.0, scale=1.0)
                    nc.gpsimd.tensor_tensor(
                        out=T1_s[:, a : a + w], in0=V_s[:, a : a + w],
                        in1=SRT1_s[:, a : a + w], op=Alu.mult)
                else:
                    nc.vector.tensor_tensor(
                        out=T1_s[:, a : a + w], in0=ps[:, :w],
                        in1=SRT1_s[:, a : a + w], op=Alu.mult)
                if i % 2 == 1 or a + w == NCOL:
                    nc.scalar.dma_start(T1P_d[:, stored : a + w],
                                        T1_s[:, stored : a + w])
                    stored = a + w
    nc.compile()
    return nc


def _prog_l3(meta):
    """T3 = (s*M3)*(relu((s*Agg(T2)/M2)@W2 + b2)@W3), fp8 half strips.
    W3 input is pre-scaled by M2*M3 on the host."""
    import concourse.mybir as mybir
    import concourse.tile as tile
    f8 = mybir.dt.float8e3
    f16 = mybir.dt.float16
    f32 = mybir.dt.float32
    Alu = mybir.AluOpType
    Act = mybir.ActivationFunctionType
    NCOL, SLOTS = meta["NCOL"], meta["SLOTS"]
    nc = _mk_bass()

    MSG_d = nc.dram_tensor("MSG", [P, SLOTS], f8, kind="ExternalInput")
    SA_d = nc.dram_tensor("SA", [P, NCOL], f16, kind="ExternalInput")
    W2_d = nc.dram_tensor("W2", [P, F2], f16, kind="ExternalInput")
    W3_d = nc.dram_tensor("W3", [F2, HID], f16, kind="ExternalInput")
    B2_d = nc.dram_tensor("B2", [F2, 1], f32, kind="ExternalInput")
    ID_d = nc.dram_tensor("ID", [P, P], f8, kind="ExternalInput")
    T3P_d = nc.dram_tensor("T3P", [P, NCOL], f8, kind="ExternalOutput")

    # deep strips of these regions are tree-summed on DVE/Pool into an fp16
    # partial; PE folds the partial into PSUM with one extra pass
    OFF = dict(_L3_OFF)

    with tile.TileContext(nc, num_cores=C) as tc:
        with (
            tc.tile_pool(name="const", bufs=1) as const,
            tc.tile_pool(name="psA", bufs=2, space="PSUM") as psAp,
            tc.tile_pool(name="ps2", bufs=4, space="PSUM") as ps2p,
            tc.tile_pool(name="ps3", bufs=2, space="PSUM") as ps3p,
        ):
            ID_s = const.tile([P, P], f8)
            nc.sync.dma_start(ID_s[:], ID_d[:])
            MSG_s = const.tile([P, SLOTS], f8)
            chunks = _msg_chunks(meta["strips"], SLOTS)
            SA_s = const.tile([P, NCOL], f16)
            W2_s = const.tile([P, F2], f16)
            W3_s = const.tile([F2, HID], f16)
            B2_s = const.tile([F2, 1], f32)
            for ci, (a, b) in enumerate(chunks):
                nc.sync.dma_start(MSG_s[:, a:b], MSG_d[:, a:b])
                if ci == 0:
                    nc.sync.dma_start(W2_s[:], W2_d[:])
                    nc.sync.dma_start(W3_s[:], W3_d[:])
                    nc.sync.dma_start(B2_s[:], B2_d[:])
                elif ci == 2:
                    nc.sync.dma_start(SA_s[:], SA_d[:])
            U_s = const.tile([P, NCOL], f16)
            H2E = const.tile([F2, NCOL], f16)
            H2O = const.tile([F2, NCOL], f16)
            T3_s = const.tile([P, NCOL], f8)
            ACC = const.tile([P, NCOL], f16)

            # PE warm-up during the first MSG DMA
            wm = psAp.tile([P, RW], f32, tag="psA")
            for i in range(8):
                nc.tensor.matmul(wm[:, :P], lhsT=ID_s[:], rhs=ID_s[:],
                                 start=(i == 0), stop=(i == 7))

            stored = [0]
            nreg = len(meta["regions"])

            def stage_a(ri):
                (r0, wr, Dr, idxs) = meta["regions"][ri]
                ps = ps_of[ri]
                cols = slice(r0, r0 + wr)
                nc.vector.tensor_tensor(out=U_s[:, cols], in0=ps[:, :wr],
                                        in1=SA_s[:, cols], op=Alu.mult)
                ps2s = []
                for hb, tp in ((0, (0, 0)), (HID, (HID, 0))):
                    ps2 = ps2p.tile([F2, RW], f32, tag="ps2")
                    nc.tensor.matmul(ps2[:, :wr],
                                     lhsT=W2_s[hb : hb + HID, :],
                                     rhs=U_s[hb : hb + HID, cols],
                                     start=True, stop=True, tile_position=tp)
                    ps2s.append(ps2)
                ps2_of[ri] = ps2s
                for ps2, H2 in zip(ps2s, (H2E, H2O)):
                    nc.scalar.activation(out=H2[:, cols], in_=ps2[:, :wr],
                                         func=Act.Relu, bias=B2_s[:],
                                         scale=1.0)

            def stage_c(ri):
                (r0, wr, Dr, idxs) = meta["regions"][ri]
                cols = slice(r0, r0 + wr)
                ps3 = ps3p.tile([P, RW], f32, tag="ps3")
                for hb, H2 in ((0, H2E), (HID, H2O)):
                    nc.tensor.matmul(ps3[hb : hb + HID, :wr], lhsT=W3_s[:],
                                     rhs=H2[:, cols], start=True, stop=True)
                nc.vector.tensor_tensor(out=T3_s[:, cols], in0=ps3[:, :wr],
                                        in1=SA_s[:, cols], op=Alu.mult)
                if ri % 2 == 1 or ri >= nreg - 3 or r0 + wr == NCOL:
                    nc.sync.dma_start(T3P_d[:, stored[0] : r0 + wr],
                                      T3_s[:, stored[0] : r0 + wr])
                    stored[0] = r0 + wr

            ps_of = {}
            ps2_of = {}
            for ri, (r0, wr, Dr, idxs) in enumerate(meta["regions"]):
                split, eng_name = OFF.get(ri, (Dr, None))
                split = min(split, Dr)
                if split < Dr:
                    eng = nc.vector if eng_name == "dve" else nc.gpsimd
                    first = True
                    for si in idxs[split:]:
                        (_, d, w, soff) = meta["strips"][si]
                        strip = MSG_s[:, soff : soff + w]
                        if first:
                            eng.tensor_copy(ACC[:, r0 : r0 + w], strip)
                            first = False
                        else:
                            eng.tensor_tensor(out=ACC[:, r0 : r0 + w],
                                              in0=ACC[:, r0 : r0 + w],
                                              in1=strip, op=Alu.add)
                ps = psAp.tile([P, RW], f32, tag="psA")
                ps_of[ri] = ps
                for j, si in enumerate(idxs[:split]):
                    (_, d, w, soff) = meta["strips"][si]
                    nc.tensor.matmul(ps[:, :w], lhsT=ID_s[:],
                                     rhs=MSG_s[:, soff : soff + w],
                                     start=(j == 0),
                                     stop=(j == Dr - 1 and split == Dr))
                if split < Dr:
                    (_, d, w, soff) = meta["strips"][idxs[split]]
                    nc.tensor.matmul(ps[:, :w], lhsT=ID_s[:],
                                     rhs=ACC[:, r0 : r0 + w],
                                     start=False, stop=True)
                if ri >= 1:
                    stage_a(ri - 1)
                if ri >= 2:
                    stage_c(ri - 2)
            stage_a(nreg - 1)
            stage_c(nreg - 2)
            stage_c(nreg - 1)
    nc.compile()
    return nc


def _prog_pool(meta):
    """Per-graph max over NPG depth-major size-ranked groups; OUTF
    [HID, GPC] f32 is features x ranked graphs (host permutes back)."""
    import concourse.mybir as mybir
    import concourse.tile as tile
    f16 = mybir.dt.float16
    f32 = mybir.dt.float32
    Alu = mybir.AluOpType
    NPG, GH = meta["NPG"], meta["GH"]
    S2G, offG, POOLW = meta["S2G"], meta["offG"], meta["POOLW"]
    nc = _mk_bass()

    MSGP_d = nc.dram_tensor("MSGP", [P, POOLW], f16, kind="ExternalInput")
    OUTF_d = nc.dram_tensor("OUTF", [HID, GPC], f32, kind="ExternalOutput")

    with tile.TileContext(nc, num_cores=C) as tc:
        with (
            tc.tile_pool(name="const", bufs=1) as const,
            tc.tile_pool(name="ps", bufs=2, space="PSUM") as psp,
        ):
            mg = const.tile([P, POOLW], f16)
            OUTF_s = const.tile([HID, GPC], f32)
            for G in range(NPG):
                nc.sync.dma_start(mg[:, offG[G] : offG[G + 1]],
                                  MSGP_d[:, offG[G] : offG[G + 1]])
            for G in range(NPG):
                g0 = offG[G]
                eng = nc.vector
                v = mg[:, g0 : offG[G + 1]].rearrange(
                    "p (d g) -> p d g", g=GH)
                cur = S2G[G]
                while cur > 1:
                    h = cur // 2
                    if cur % 2:
                        eng.tensor_tensor(
                            out=v[:, 0, :], in0=v[:, 0, :],
                            in1=v[:, cur - 1, :], op=Alu.max)
                    eng.tensor_tensor(
                        out=v[:, 0:h, :], in0=v[:, 0:h, :],
                        in1=v[:, h : 2 * h, :], op=Alu.max)
                    cur = h
                bot = psp.tile([HID, GH], f32, tag="bot")
                nc.vector.tensor_copy(bot[:], mg[HID:P, g0 : g0 + GH])
                nc.vector.tensor_tensor(
                    out=OUTF_s[:, G * GH : (G + 1) * GH],
                    in0=mg[0:HID, g0 : g0 + GH],
                    in1=bot[:], op=Alu.max)
            nc.sync.dma_start(OUTF_d[:], OUTF_s[:])
    nc.compile()
    return nc


# --------------------------------------------------------------------------
# Entry point
# --------------------------------------------------------------------------

_RUN_KWARGS = {}
_EXEC_NS = []
_PROFILE = False


def _stage_msgs(T_full, srcmap_c):
    """[N+1, HID] table + [2, SLOTS] slot->row map -> [128, SLOTS]."""
    top = T_full[srcmap_c[0]].T      # [64, SLOTS]
    bot = T_full[srcmap_c[1]].T
    return np.ascontiguousarray(np.concatenate([top, bot], axis=0))


def _assemble(prep, parts_E, parts_O, dtype):
    """Per-core [64, NCOL] half strips -> full [N+1, HID] table."""
    T_full = np.zeros((N + 1, HID), dtype)
    for c in range(C):
        tops, bots = prep["tops"][c], prep["bots"][c]
        mE, mO = tops >= 0, bots >= 0
        T_full[tops[mE]] = parts_E[c][:, mE].T
        T_full[bots[mO]] = parts_O[c][:, mO].T
    return T_full


def kernel(data, edge_index, batch, W1, b1, W2, b2, W3, b3):
    from concourse.bass_utils import run_bass_kernel_spmd

    data = np.asarray(data, dtype=np.float32)
    edge_index = np.asarray(edge_index, dtype=np.int32)
    batch_np = np.asarray(batch, dtype=np.int32)

    prep = _host_prep(edge_index, batch_np)
    meta = prep["meta"]
    NCOL = meta["NCOL"]

    IDENT8 = np.eye(P, dtype=F8)
    W1f = np.asarray(W1, np.float32).astype(F16)            # [128, 64]
    W2f = np.asarray(W2, np.float32).astype(F16)            # [64, 128]
    W3f = (np.asarray(W3, np.float32) * (M2 * M3)).astype(F16)
    B1K = (np.tile(np.asarray(b1, np.float32), 2) * (M1 * M2))[:, None].copy()
    B2r = np.asarray(b2, np.float32)[:, None].copy()
    B3r = np.tile(np.asarray(b3, np.float32), 2)[:, None].copy()

    Xx = np.concatenate([data, np.zeros((1, IN_DIM), np.float32)], axis=0)
    XT = np.empty((C, IN_DIM, 2 * NCOL), F16)
    for c in range(C):
        XT[c, :, :NCOL] = Xx[prep["topsx"][c]].T
        XT[c, :, NCOL:] = Xx[prep["botsx"][c]].T

    cores = list(range(C))
    del _EXEC_NS[:]

    def run(nc, in_maps):
        if _PROFILE:
            from concourse.timeline_sim import TimelineSim
            _EXEC_NS.append(TimelineSim(nc, require_finite=False).simulate())
        res = run_bass_kernel_spmd(nc, in_maps, cores, **_RUN_KWARGS)
        if res.exec_time_ns is not None:
            _EXEC_NS.append(res.exec_time_ns)
        return res.results

    # ---- L1: T1 = (s*M1) * (X @ W1) ----
    r1 = run(_prog_l1(meta),
             [{"XT": np.ascontiguousarray(XT[c]), "W1": W1f,
               "SRT1": np.ascontiguousarray(prep["SRT1"][c])}
              for c in range(C)])
    T1 = _assemble(prep,
                   [np.asarray(r1[c]["T1P"])[0:HID] for c in range(C)],
                   [np.asarray(r1[c]["T1P"])[HID:P] for c in range(C)], F8)

    # ---- L2: T2 = (s*M2)*relu(s*Agg(T1)/M1 + b1) ----
    r2 = run(_prog_agg(meta, 2),
             [{"MSG": _stage_msgs(T1, prep["srcmap"][c]),
               "SA": np.ascontiguousarray(prep["SA2"][c]), "BK": B1K,
               "ID": IDENT8}
              for c in range(C)])
    T2 = _assemble(prep,
                   [np.asarray(r2[c]["OUT"])[0:HID] for c in range(C)],
                   [np.asarray(r2[c]["OUT"])[HID:P] for c in range(C)], F8)

    # ---- L3: T3 = (s*M3)*(relu((s*Agg(T2)/M2)@W2 + b2)@W3) ----
    r3 = run(_prog_l3(meta),
             [{"MSG": _stage_msgs(T2, prep["srcmap"][c]),
               "SA": np.ascontiguousarray(prep["SA3"][c]),
               "W2": np.concatenate([W2f, W2f], axis=0), "W3": W3f,
               "B2": B2r, "ID": IDENT8}
              for c in range(C)])
    T3 = _assemble(prep,
                   [np.asarray(r3[c]["T3P"])[0:HID] for c in range(C)],
                   [np.asarray(r3[c]["T3P"])[HID:P] for c in range(C)], F8)

    # ---- L4: H3 = s*Agg(T3)/M3 + b3 ----
    r4 = run(_prog_agg(meta, 4),
             [{"MSG": _stage_msgs(T3, prep["srcmap"][c]),
               "SA": np.ascontiguousarray(prep["SA4"][c]), "BK": B3r,
               "ID": IDENT8}
              for c in range(C)])
    H3 = _assemble(prep,
                   [np.asarray(r4[c]["OUT"])[0:HID] for c in range(C)],
                   [np.asarray(r4[c]["OUT"])[HID:P] for c in range(C)], F16)
    H3[N] = np.float16(-60000.0)     # pad row for the pool staging

    # ---- L5: per-graph max pool ----
    r5 = run(_prog_pool(meta),
             [{"MSGP": _stage_msgs(H3, prep["poolmap"][c])}
              for c in range(C)])
    out = np.empty((N_GRAPHS, HID), np.float32)
    for c in range(C):
        of = np.asarray(r5[c]["OUTF"]).astype(np.float32).T   # ranked graphs
        out[c * GPC + prep["grank"][c]] = of
    out[prep["cnt"].reshape(-1) == 0] = -np.inf
    return out
